# revision 1
# baseline (speedup 1.0000x reference)
"""Trainium2 Bass kernel for nn_Attention_15410342658774.

Location-sensitive monotonic attention + additive self-attention
(Tacotron-style), B=64, T=1000, E=EL=512, RNN=1024, AD=128.

Strategy: pure data parallel across 8 NeuronCores (8 batch rows each,
weights replicated).  The host pre-transposes `memory`/`self_memory` to
[B, E, T] (bf16), so each per-row tile streams from HBM exactly once as
a single contiguous-run DMA in the E-on-partitions layout that feeds:
  pm  = W @ mem.T        (PE, contraction over E, bf16)
  e   = v . tanh(pq + proc_attn + pm)   (ACT tanh + PE contraction over AD)
  ctx = sum_t align[t] * memT[:, t]     (DVE scalar_tensor_tensor accum
                                         against a PE-broadcast align row)
No second pass over memory is needed.  All row-wise math runs on the full
local batch (8 rows) at partitions 0..7 — compute engines cannot start at
unaligned partitions — and DMA moves rows into place.

Hardware constraints baked in (found the hard way):
  - matmul operands/outputs must start at partition 0/32/64
  - DMA access patterns: at most 3 [step,count] dims, innermost step 1
  - TENSOR_TENSOR_REDUCE crashes the exec unit; scalar_tensor_tensor's
    accum_out is the working per-partition reduction
  - one sync-wait per matmul at codegen → Bacc (wait splitting) + few
    DMA writers per consumed tile
"""

import dataclasses as _dc
import sys

import numpy as np

_TRN = "/opt/trn_rl_repo"
if _TRN not in sys.path:
    sys.path.insert(0, _TRN)

from contextlib import ExitStack

import ml_dtypes

import concourse.bacc as bacc
import concourse.bass as bass
import concourse.mybir as mybir
from concourse.bass_utils import run_bass_kernel_spmd
from concourse.masks import make_identity
from concourse.tile import TileContext

B, T = 64, 1000
E, EL, RNN, AD = 512, 512, 1024, 128
NF, K = 32, 31
PAD = (K - 1) // 2
NCORES = 8
BL = B // NCORES  # 8 batch rows per core
F32 = mybir.dt.float32
BF16 = mybir.dt.bfloat16
AF = mybir.ActivationFunctionType
ALU = mybir.AluOpType
AX = mybir.AxisListType
SEGS = [(0, 512), (512, 488)]  # T split at the 512-float PSUM bank boundary

# output packing: [context(E) | alignments(T) | u_new(1) | cum_new(T) | ctx2(EL) | w2(T)]
CTX0 = 0
ALIGN0 = E
UN0 = E + T
CUM0 = E + T + 1
CTX2_0 = E + 2 * T + 1
W2_0 = E + EL + 2 * T + 1
OUT_W = E + EL + 3 * T + 1  # 4025

MEMT_BUFS = 11  # 8 live per phase + cross-phase prefetch
TANH_BUFS = 3


def build_nc(finalize: bool = True, repeat: int = 1) -> bass.Bass:
    nc = bacc.Bacc()

    q_d = nc.declare_dram_parameter("query", [BL, RNN], F32, isOutput=False)
    # pre-transposed [E, T] per row, bf16 (host-prepared)
    mem_d = nc.declare_dram_parameter("memory", [BL, E, T], BF16, isOutput=False)
    smem_d = nc.declare_dram_parameter("self_memory", [BL, EL, T], BF16, isOutput=False)
    aw_d = nc.declare_dram_parameter("attention_weights", [BL, T], BF16, isOutput=False)
    awc_d = nc.declare_dram_parameter(
        "attention_weights_cum", [BL, T], F32, isOutput=False
    )
    awcb_d = nc.declare_dram_parameter("awc_bf", [BL, T], BF16, isOutput=False)
    al_d = nc.declare_dram_parameter("alpha", [BL, T], F32, isOutput=False)
    u_d = nc.declare_dram_parameter("u", [BL, 1], F32, isOutput=False)
    mW_d = nc.declare_dram_parameter("memory_W", [AD, E], BF16, isOutput=False)
    qW_d = nc.declare_dram_parameter("query_W", [AD, RNN], F32, isOutput=False)
    vW_d = nc.declare_dram_parameter("v_W", [1, AD], F32, isOutput=False)
    cW_d = nc.declare_dram_parameter("loc_conv_W", [NF, 2, K], F32, isOutput=False)
    dW_d = nc.declare_dram_parameter("loc_dense_W", [AD, NF], F32, isOutput=False)
    taW_d = nc.declare_dram_parameter("ta_W", [1, E + RNN], F32, isOutput=False)
    tab_d = nc.declare_dram_parameter("ta_b", [1, 1], F32, isOutput=False)
    smW_d = nc.declare_dram_parameter("self_memory_W", [AD, EL], BF16, isOutput=False)
    sqW_d = nc.declare_dram_parameter("self_query_W", [AD, RNN], F32, isOutput=False)
    svW_d = nc.declare_dram_parameter("self_v_W", [1, AD], F32, isOutput=False)
    out_d = nc.declare_dram_parameter("out", [BL, OUT_W], F32, isOutput=True)

    with ExitStack() as ctx:
        tc = ctx.enter_context(TileContext(nc))
        cpool = ctx.enter_context(tc.tile_pool(name="const", bufs=1))
        mpool = ctx.enter_context(tc.tile_pool(name="mem", bufs=MEMT_BUFS))
        tpool = ctx.enter_context(tc.tile_pool(name="tanhp", bufs=TANH_BUFS))
        cspool = ctx.enter_context(tc.tile_pool(name="convsp", bufs=2))
        scpool = ctx.enter_context(tc.tile_pool(name="scrp", bufs=2))
        rtp = ctx.enter_context(tc.tile_pool(name="rtp", bufs=1))
        ppm = ctx.enter_context(tc.tile_pool(name="ppm", bufs=2, space="PSUM"))
        pbc = ctx.enter_context(tc.tile_pool(name="pbc", bufs=1, space="PSUM"))
        ppe = ctx.enter_context(tc.tile_pool(name="ppe", bufs=2, space="PSUM"))

        # ---------------- phase-1 memory loads first (critical stream) ----
        def load_memT(mem_dram, b, eng):
            memT = mpool.tile([128, 4 * T], BF16, tag="memT", name="memT")
            # one DMA per row: [E, T] contiguous rows -> [128, (ec t)]
            eng.dma_start(
                out=memT[:].rearrange("p (c t) -> p c t", c=4),
                in_=mem_dram[b : b + 1].rearrange("b (c p) t -> (b p) c t", p=128),
            )
            return memT

        memTs_p1 = {
            b: load_memT(mem_d, b, (nc.sync, nc.scalar)[b % 2]) for b in range(BL)
        }

        # ---------------- constant / weight loads ----------------
        ident = cpool.tile([128, 128], F32, tag="ident")
        make_identity(nc, ident[:])
        ident_bf = cpool.tile([128, 128], BF16, tag="ident_bf")
        nc.scalar.activation(ident_bf[:], ident[:], AF.Copy)

        ldp = ctx.enter_context(tc.tile_pool(name="ldp", bufs=1))

        def load_wT(dram, ncols, name, dt):
            # [AD, C] (torch Linear layout) -> W.T chunks via ONE natural
            # (contiguous) DMA + PE transposes; strided per-chunk DMAs cost
            # ~1.3us issue each on the DGE path.
            nat = ldp.tile([128, ncols], dt, tag="wnat", name="wnat")
            nc.scalar.dma_start(out=nat[:], in_=dram[:])
            t = cpool.tile([128, ncols], dt, tag=name)
            idn = ident if dt == F32 else ident_bf
            for c in range(ncols // 128):
                tp = ppm.tile([128, 128], dt, tag="pm", name="wtp")
                nc.tensor.transpose(tp[:], nat[:, c * 128 : (c + 1) * 128], idn[:])
                nc.scalar.activation(t[:, c * 128 : (c + 1) * 128], tp[:], AF.Copy)
            return t

        mWT = load_wT(mW_d, E, "mWT", BF16)
        qWT = load_wT(qW_d, RNN, "qWT", F32)
        smWT = load_wT(smW_d, EL, "smWT", BF16)
        sqWT = load_wT(sqW_d, RNN, "sqWT", F32)

        q_sb = cpool.tile([BL, RNN], F32, tag="q_sb")
        nc.scalar.dma_start(out=q_sb[:], in_=q_d[:])
        qT = cpool.tile([128, 8 * BL], F32, tag="qT")  # cols (rchunk, b)
        for c in range(8):
            tp = ppm.tile([128, BL], F32, tag="pm", name="qtp")
            nc.tensor.transpose(
                tp[:], q_sb[:, c * 128 : (c + 1) * 128], ident[0:BL, 0:BL]
            )
            nc.scalar.activation(qT[:, c * BL : (c + 1) * BL], tp[:], AF.Copy)

        convWT_f = cpool.tile([2 * K, NF], F32, tag="convWT_f")  # [(c k), o]
        nc.scalar.dma_start(out=convWT_f[:], in_=cW_d[:].rearrange("o c k -> (c k) o"))
        convWT = cpool.tile([2 * K, NF], BF16, tag="convWT")
        nc.scalar.activation(convWT[:], convWT_f[:], AF.Copy)
        ldWT_f = cpool.tile([NF, AD], F32, tag="ldWT_f")
        nc.scalar.dma_start(out=ldWT_f[:], in_=dW_d[:].rearrange("a f -> f a"))
        ldWT = cpool.tile([NF, AD], BF16, tag="ldWT")
        nc.scalar.activation(ldWT[:], ldWT_f[:], AF.Copy)

        ones1 = cpool.tile([1, 128], F32, tag="ones1")
        nc.vector.memset(ones1[:], 1.0)

        # sel[:, b*128:(b+1)*128] is an [8, 128] selector whose row b is all
        # ones: bc = sel_b.T @ w_rt broadcasts w row b across 128 partitions
        # in a single matmul (no staging DMA in the context hot path).
        ones_d = nc.dram_tensor("ones_row", [1, 128], F32)
        nc.sync.dma_start(out=ones_d[:], in_=ones1[:])
        sel = cpool.tile([BL, BL * 128], F32, tag="sel")
        nc.vector.memset(sel[:], 0.0)
        for b in range(BL):
            nc.sync.dma_start(
                out=sel[b : b + 1, b * 128 : (b + 1) * 128], in_=ones_d[:]
            )

        # vmat[:, bi*BL + j] = v if j == bi else 0 — lets the e = v.tanh(...)
        # contraction for row bi accumulate into PSUM row bi (matmul PSUM
        # outputs must start at partition 0).
        v_sb = cpool.tile([128, 2], F32, tag="v_sb")
        nc.scalar.dma_start(out=v_sb[:, 0:1], in_=vW_d[:].rearrange("o a -> a o"))
        nc.scalar.dma_start(out=v_sb[:, 1:2], in_=svW_d[:].rearrange("o a -> a o"))

        def masked_v(col, name):
            t = cpool.tile([128, BL * BL], BF16, tag=name)
            nc.vector.memset(t[:], 0.0)
            for bi in range(BL):
                c = bi * BL + bi
                nc.scalar.activation(
                    t[:, c : c + 1], v_sb[:, col : col + 1], AF.Copy
                )
            return t

        vmat = masked_v(0, "vmat")
        svmat = masked_v(1, "svmat")

        # im2col of [attention_weights; attention_weights_cum] for the
        # location conv: rows (c, k), cols (b, t); zero-padded edges.  Bounce
        # the zero-padded rows through DRAM so the sliding window is 2 DMAs
        # (one per channel) — consumers may carry only a few waits.
        TP = T + 2 * PAD
        pad_d = nc.dram_tensor("awc_pad", [2 * BL, TP], BF16)
        zero_sb = cpool.tile([2 * BL, 16], BF16, tag="zero_sb")
        nc.vector.memset(zero_sb[:], 0.0)
        nc.gpsimd.dma_start(out=pad_d[:, 0:PAD], in_=zero_sb[:, 0:PAD])
        nc.gpsimd.dma_start(out=pad_d[:, TP - PAD : TP], in_=zero_sb[:, 0:PAD])
        nc.gpsimd.dma_start(out=pad_d[0:BL, PAD : PAD + T], in_=aw_d[:])
        nc.gpsimd.dma_start(out=pad_d[BL : 2 * BL, PAD : PAD + T], in_=awcb_d[:])
        im2 = cpool.tile([2 * K, BL * T], BF16, tag="im2")
        for c in range(2):
            src = _dc.replace(
                pad_d[:], ap=[[1, K], [TP, BL], [1, T]], offset=c * BL * TP
            )
            nc.gpsimd.dma_start(
                out=im2[c * K : (c + 1) * K, :].rearrange("k (b t) -> k b t", b=BL),
                in_=src,
            )

        # per-row scalars: 4=s1 5=ta_b 6=s2 7=scratch
        cols = cpool.tile([BL, 8], F32, tag="cols")
        nc.sync.dma_start(
            out=cols[:, 5:6], in_=_dc.replace(tab_d[:], ap=[[0, BL], [1, 1]])
        )
        taWb = cpool.tile([BL, E + RNN], F32, tag="taWb")
        nc.sync.dma_start(
            out=taWb[:], in_=_dc.replace(taW_d[:], ap=[[0, BL], [1, E + RNN]])
        )

        # context / u_new staging for the whole local batch (partitions 0..BL)
        out_tile = cpool.tile([BL, OUT_W], F32, tag="out_tile")
        # u_new scratch lives in out_tile columns that never reach DRAM
        # (alignments/cum/w2 sections are written to DRAM from row tiles)
        scr8 = out_tile[0:BL, CUM0 : CUM0 + RNN]

        # ---------------- query projections (pq, spq) ----------------
        def project_query(wT, name):
            ps = ppm.tile([128, BL], F32, tag="pm", name="pq_ps")
            for rc in range(8):
                nc.tensor.matmul(
                    ps[:],
                    lhsT=wT[:, rc * 128 : (rc + 1) * 128],
                    rhs=qT[:, rc * BL : (rc + 1) * BL],
                    start=(rc == 0),
                    stop=(rc == 7),
                )
            sb = cpool.tile([128, BL], F32, tag=name)
            nc.scalar.activation(sb[:], ps[:], AF.Copy)
            return sb

        pq_sb = project_query(qWT, "pq_sb")
        spq_sb = project_query(sqWT, "spq_sb")

        # ---------------- one attention phase ----------------
        def attn_phase(mem_dram, wT, pq, v, with_loc, ctx_off, wout_off, preload=None):
            ctxT = cpool.tile([128, 4 * BL], F32, tag="ctxT", name="ctxT")
            e_ps = ppe.tile([BL, T], F32, tag="pe", name="e_ps")
            memTs = {}
            for b in range(BL):
                if preload is not None:
                    memT = preload[b]
                else:
                    memT = load_memT(mem_dram, b, (nc.sync, nc.scalar)[b % 2])
                memTs[b] = memT
                if with_loc:
                    conv_s = cspool.tile([NF, T], BF16, tag="convs", name="conv_s")
                    cps = pbc.tile([NF, T], F32, tag="bc", name="cps")
                    for t0, tl in SEGS:
                        nc.tensor.matmul(
                            cps[:, t0 : t0 + tl],
                            lhsT=convWT[:],
                            rhs=im2[:, b * T + t0 : b * T + t0 + tl],
                            start=True,
                            stop=True,
                        )
                    nc.scalar.activation(conv_s[:], cps[:], AF.Copy)
                th = tpool.tile([128, T], BF16, tag="tanh", name="th")
                for si, (t0, tl) in enumerate(SEGS):
                    pm = ppm.tile([128, 512], F32, tag="pm", name="pm")
                    for ec in range(4):
                        nc.tensor.matmul(
                            pm[:, 0:tl],
                            lhsT=wT[:, ec * 128 : (ec + 1) * 128],
                            rhs=memT[:, ec * T + t0 : ec * T + t0 + tl],
                            start=(ec == 0),
                            stop=(ec == 3 and not with_loc),
                        )
                    if with_loc:
                        nc.tensor.matmul(
                            pm[:, 0:tl],
                            lhsT=ldWT[:],
                            rhs=conv_s[:, t0 : t0 + tl],
                            start=False,
                            stop=True,
                        )
                    nc.scalar.activation(
                        th[:, t0 : t0 + tl], pm[:, 0:tl], AF.Tanh,
                        bias=pq[:, b : b + 1],
                    )
                    nc.tensor.matmul(
                        e_ps[:, t0 : t0 + tl],
                        lhsT=v[:, b * BL : (b + 1) * BL],
                        rhs=th[:, t0 : t0 + tl],
                        start=(b == 0),
                        stop=(b == BL - 1),
                        skip_group_check=True,
                    )

            # ---- row-wise (DVE/ACT) section on partitions 0..BL ----
            sig_rt = rtp.tile([BL, T], F32, tag="sig_rt", name="sig_rt")
            w_rt = rtp.tile([BL, T], F32, tag="w_rt", name="w_rt")
            colsr = rtp.tile([BL, 8], F32, tag="colsr", name="colsr")
            nc.scalar.activation(sig_rt[:], e_ps[:], AF.Sigmoid)
            nc.vector.reduce_sum(out=colsr[:, 6:7], in_=sig_rt[:], axis=AX.X)
            nc.vector.reciprocal(colsr[:, 7:8], colsr[:, 6:7])
            if with_loc:
                awc_rt = rtp.tile([BL, T], F32, tag="awc_rt", name="awc_rt")
                nc.sync.dma_start(out=awc_rt[:], in_=awc_d[:])
                alpha_rt = rtp.tile([BL, T], F32, tag="alpha_rt", name="alpha_rt")
                nc.sync.dma_start(out=alpha_rt[:], in_=al_d[:])
                nc.sync.dma_start(out=colsr[:, 0:1], in_=u_d[:])
                base_rt = rtp.tile([BL, T], F32, tag="base_rt", name="base_rt")
                shift_rt = rtp.tile([BL, T], F32, tag="shift_rt", name="shift_rt")
                anew_rt = rtp.tile([BL, T], F32, tag="anew_rt", name="anew_rt")
                # cum_new = awc + sig/sum(sig) in one fused op
                nc.vector.scalar_tensor_tensor(
                    out=anew_rt[:],
                    in0=sig_rt[:],
                    scalar=colsr[:, 7:8],
                    in1=awc_rt[:],
                    op0=ALU.mult,
                    op1=ALU.add,
                )
                nc.sync.dma_start(out=out_d[:, CUM0 : CUM0 + T], in_=anew_rt[:])
                # monotonic alpha recurrence; the sigmoid-normalizing scalar
                # cancels in alignments = x/sum(x), so the chain runs off raw
                # sig (parallel to the sum/reciprocal above):
                #   base = (1-u)*alpha + u*shift(alpha)
                #   w    = (base + 1e-8)*sig, normalized
                nc.vector.tensor_scalar(
                    out=colsr[:, 1:2],
                    in0=colsr[:, 0:1],
                    scalar1=-1.0,
                    scalar2=1.0,
                    op0=ALU.mult,
                    op1=ALU.add,
                )
                nc.vector.memset(shift_rt[:, 0:1], 0.0)
                nc.vector.tensor_scalar_mul(
                    shift_rt[:, 1:T], alpha_rt[:, 0 : T - 1], colsr[:, 0:1]
                )
                nc.vector.scalar_tensor_tensor(
                    out=base_rt[:],
                    in0=alpha_rt[:],
                    scalar=colsr[:, 1:2],
                    in1=shift_rt[:],
                    op0=ALU.mult,
                    op1=ALU.add,
                )
                nc.vector.scalar_tensor_tensor(
                    out=base_rt[:],
                    in0=base_rt[:],
                    scalar=1e-8,
                    in1=sig_rt[:],
                    op0=ALU.add,
                    op1=ALU.mult,
                )
                nc.vector.reduce_sum(out=colsr[:, 2:3], in_=base_rt[:], axis=AX.X)
                nc.vector.reciprocal(colsr[:, 3:4], colsr[:, 2:3])
                nc.vector.tensor_scalar_mul(w_rt[:], base_rt[:], colsr[:, 3:4])
            else:
                nc.vector.tensor_scalar_mul(w_rt[:], sig_rt[:], colsr[:, 7:8])
            nc.sync.dma_start(out=out_d[:, wout_off : wout_off + T], in_=w_rt[:])

            # ---- context: ctxT[:, ec*BL+b] = sum_t w[t] * memT[:, ec, t] ----
            # (TENSOR_TENSOR_REDUCE crashes the exec unit on this runtime;
            # scalar_tensor_tensor's accum_out is the working reduction.)
            for b in range(BL):
                bc = pbc.tile([128, T], F32, tag="bc", name="bc")
                for t0, tl in SEGS:
                    nc.tensor.matmul(
                        bc[:, t0 : t0 + tl],
                        lhsT=sel[:, b * 128 : (b + 1) * 128],
                        rhs=w_rt[:, t0 : t0 + tl],
                        start=True,
                        stop=True,
                    )
                # bounce to SBUF so the single PSUM slot frees right away
                # (and bf16 SBUF operands put the DVE in its fast mode)
                bc_sb = scpool.tile([128, T], BF16, tag="bcsb", name="bc_sb", bufs=4)
                nc.scalar.activation(bc_sb[:], bc[:], AF.Copy)
                for ec in range(4):
                    cc = ec * BL + b
                    scr = scpool.tile([128, T], BF16, tag="scr", name="scr")
                    nc.vector.scalar_tensor_tensor(
                        out=scr[:],
                        in0=memTs[b][:, ec * T : (ec + 1) * T],
                        scalar=1.0,
                        in1=bc_sb[:],
                        op0=ALU.mult,
                        op1=ALU.mult,
                        accum_out=ctxT[:, cc : cc + 1],
                    )

            # ---- transpose ctxT [128, (ec b)] -> out rows [b, E] ----
            for ec in range(4):
                tp = ppm.tile([BL, 128], F32, tag="pm", name="tp")
                nc.tensor.transpose(tp[:], ctxT[:, ec * BL : (ec + 1) * BL], ident[:])
                nc.scalar.activation(
                    out_tile[0:BL, ctx_off + ec * 128 : ctx_off + (ec + 1) * 128],
                    tp[:],
                    AF.Copy,
                )

        for _rep in range(repeat):
            attn_phase(mem_d, mWT, pq_sb, vmat, True, CTX0, ALIGN0,
                       preload=memTs_p1 if _rep == 0 else None)

            # u_new = sigmoid([context, query] @ ta_W.T + ta_b)
            nc.vector.scalar_tensor_tensor(
                out=scr8[:, 0:E],
                in0=out_tile[0:BL, CTX0 : CTX0 + E],
                scalar=1.0,
                in1=taWb[:, 0:E],
                op0=ALU.mult,
                op1=ALU.mult,
                accum_out=cols[:, 4:5],
            )
            nc.vector.scalar_tensor_tensor(
                out=scr8[:, 0:RNN],
                in0=q_sb[:],
                scalar=1.0,
                in1=taWb[:, E : E + RNN],
                op0=ALU.mult,
                op1=ALU.mult,
                accum_out=cols[:, 6:7],
            )
            nc.vector.tensor_add(cols[:, 7:8], cols[:, 4:5], cols[:, 6:7])
            nc.scalar.activation(
                out_tile[0:BL, UN0 : UN0 + 1], cols[:, 7:8], AF.Sigmoid, bias=cols[:, 5:6]
            )

            attn_phase(smem_d, smWT, spq_sb, svmat, False, CTX2_0, W2_0)

            # context, u_new, ctx2 live in out_tile; alignments/cum_new/w2 were
            # DMA'd to DRAM directly from the row tiles.
            nc.sync.dma_start(out=out_d[:, CTX0 : CTX0 + E], in_=out_tile[:, CTX0 : CTX0 + E])
            nc.sync.dma_start(out=out_d[:, UN0 : UN0 + 1], in_=out_tile[:, UN0 : UN0 + 1])
            nc.sync.dma_start(
                out=out_d[:, CTX2_0 : CTX2_0 + EL], in_=out_tile[:, CTX2_0 : CTX2_0 + EL]
            )

    if finalize:
        nc.finalize()
    return nc


_NC = None
RUN_KWARGS: dict = {}   # test harness can set {"trace": True}
LAST_RESULT = None      # BassKernelResults of the most recent kernel() call


def _get_nc():
    global _NC
    if _NC is None:
        _NC = build_nc()
    return _NC


def make_in_map(shard: dict) -> dict:
    """Device in_map for ONE core's shard (keys as in setup_inputs)."""
    f = lambda k: np.ascontiguousarray(np.asarray(shard[k], dtype=np.float32))
    bf = ml_dtypes.bfloat16
    return {
        "query": f("query"),
        "memory": np.ascontiguousarray(f("memory").transpose(0, 2, 1).astype(bf)),
        "self_memory": np.ascontiguousarray(
            f("self_memory").transpose(0, 2, 1).astype(bf)
        ),
        "attention_weights": f("attention_weights").astype(bf),
        "attention_weights_cum": f("attention_weights_cum"),
        "awc_bf": f("attention_weights_cum").astype(bf),
        "alpha": f("alpha"),
        "u": f("u"),
        "memory_W": f("memory_W").astype(bf),
        "query_W": f("query_W"),
        "v_W": f("v_W"),
        "loc_conv_W": f("loc_conv_W"),
        "loc_dense_W": f("loc_dense_W"),
        "ta_W": f("ta_W"),
        "ta_b": f("ta_b").reshape(1, 1),
        "self_memory_W": f("self_memory_W").astype(bf),
        "self_query_W": f("self_query_W"),
        "self_v_W": f("self_v_W"),
    }


def kernel(**inputs) -> np.ndarray:
    f = lambda k: np.ascontiguousarray(np.asarray(inputs[k], dtype=np.float32))
    bf = ml_dtypes.bfloat16
    rep = {
        "memory_W": f("memory_W").astype(bf),
        "query_W": f("query_W"),
        "v_W": f("v_W"),
        "loc_conv_W": f("loc_conv_W"),
        "loc_dense_W": f("loc_dense_W"),
        "ta_W": f("ta_W"),
        "ta_b": f("ta_b").reshape(1, 1),
        "self_memory_W": f("self_memory_W").astype(bf),
        "self_query_W": f("self_query_W"),
        "self_v_W": f("self_v_W"),
    }
    mem_t = np.ascontiguousarray(
        f("memory").transpose(0, 2, 1).astype(bf)
    )  # [B, E, T] bf16
    smem_t = np.ascontiguousarray(f("self_memory").transpose(0, 2, 1).astype(bf))
    aw_bf = f("attention_weights").astype(bf)
    awc = f("attention_weights_cum")
    awc_bf = awc.astype(bf)
    q = f("query")
    alpha = f("alpha")
    u = f("u")
    in_maps = []
    for i in range(NCORES):
        sl = slice(i * BL, (i + 1) * BL)
        m = dict(rep)
        m["query"] = q[sl]
        m["memory"] = mem_t[sl]
        m["self_memory"] = smem_t[sl]
        m["attention_weights"] = aw_bf[sl]
        m["attention_weights_cum"] = awc[sl]
        m["awc_bf"] = awc_bf[sl]
        m["alpha"] = alpha[sl]
        m["u"] = u[sl]
        in_maps.append(m)
    global LAST_RESULT
    res = run_bass_kernel_spmd(
        _get_nc(), in_maps, core_ids=list(range(NCORES)), **RUN_KWARGS
    )
    LAST_RESULT = res
    return np.concatenate([res.results[i]["out"] for i in range(NCORES)], axis=0)



# revision 37
# speedup vs baseline: 1.9026x; 1.9026x over previous
"""Trainium2 Bass kernel for nn_Attention_15410342658774 (v2).

Location-sensitive monotonic attention + additive self-attention
(Tacotron-style), B=64, T=1000, E=EL=512, RNN=1024, AD=128.

Pure data parallel across 8 NeuronCores (8 batch rows each, weights
replicated).  Host pre-transposes `memory`/`self_memory` to [B, E, T]
and casts to fp8e4 (halves the HBM stream vs bf16 and enables DoubleRow
matmuls); weight matrices ride along as fp8e4 scaled by 64 (folded back
out via the tanh activation's input scale).

v2 changes vs the 162us baseline (cost-model-driven):
  - fp8e4 memory stream: DMA_ENGINES floor 56us -> ~28us; DoubleRow
    fp8 matmuls for pm (2 k-tiles per instruction at 0.5 cyc/row).
  - All big DMAs on dedicated queues (sync for phase 1 + weights,
    gpsimd for phase 2 + im2col) - never on the ACT/DVE/PE queues,
    since a dma_start occupies its queue for the whole transfer.
  - ctx accumulation (the 82us DVE hotspot; scalar_tensor_tensor has
    no DVE fast mode) split DVE/Pool: DVE rows read the bc broadcast
    straight from PSUM, Pool rows get an ACT-copied bf16 SBUF view
    (gpsimd cannot touch PSUM).
  - bc broadcast matmuls in bf16 (they were f32 = 4 cyc/row on PE).
  - reductions fused into producers via accum_out (sigmoid-sum,
    alpha-chain sum).
  - PSUM budget reworked to exactly 8 banks: pm 2 + bc 2x2 + e-segs 2.

Hardware constraints baked in (from the v1 session + cost model):
  - matmul operands/outputs must start at partition 0/32/64; PSUM
    matmul outputs must not cross a 2KB bank.
  - DMA access patterns: at most 3 [step,count] dims, innermost step 1.
  - TENSOR_TENSOR_REDUCE crashes the exec unit; scalar_tensor_tensor's
    accum_out is the working per-partition reduction.
  - gpsimd (Pool) engine: SBUF only, no PSUM access.
"""

import dataclasses as _dc
import sys

import numpy as np

_TRN = "/opt/trn_rl_repo"
if _TRN not in sys.path:
    sys.path.insert(0, _TRN)

from contextlib import ExitStack

import ml_dtypes

import concourse.bacc as bacc
import concourse.bass as bass
import concourse.mybir as mybir
from concourse.bass_utils import run_bass_kernel_spmd
from concourse.masks import make_identity
from concourse.tile import TileContext

B, T = 64, 1000
E, EL, RNN, AD = 512, 512, 1024, 128
NF, K = 32, 31
PAD = (K - 1) // 2
NCORES = 8
BL = B // NCORES  # 8 batch rows per core
F32 = mybir.dt.float32
BF16 = mybir.dt.bfloat16
F8 = mybir.dt.float8e4
AF = mybir.ActivationFunctionType
ALU = mybir.AluOpType
AX = mybir.AxisListType
PM_DR = mybir.MatmulPerfMode.DoubleRow
SEGS = [(0, 512), (512, 488)]  # T split at the 512-float PSUM bank boundary
WS = 64.0  # fp8 weight pre-scale (exact power of two)

# output packing: [context(E) | alignments(T) | u_new(1) | cum_new(T) | ctx2(EL) | w2(T)]
CTX0 = 0
ALIGN0 = E
UN0 = E + T
CUM0 = E + T + 1
CTX2_0 = E + 2 * T + 1
W2_0 = E + EL + 2 * T + 1
OUT_W = E + EL + 3 * T + 1  # 4025

DVE_ROWS = (0, 2, 4, 6, 7)  # ctx rows on DVE; rest on Pool (TT+ACT accum)


def build_nc(finalize: bool = True, repeat: int = 1) -> bass.Bass:
    nc = bacc.Bacc()

    q_d = nc.declare_dram_parameter("query", [BL, RNN], F32, isOutput=False)
    # pre-transposed [E, T] per row, fp8e4 (host-prepared)
    mem_d = nc.declare_dram_parameter("memory", [BL, E, T], F8, isOutput=False)
    smem_d = nc.declare_dram_parameter("self_memory", [BL, EL, T], F8, isOutput=False)
    aw_d = nc.declare_dram_parameter("attention_weights", [BL, T], BF16, isOutput=False)
    awc_d = nc.declare_dram_parameter(
        "attention_weights_cum", [BL, T], F32, isOutput=False
    )
    awcb_d = nc.declare_dram_parameter("awc_bf", [BL, T], BF16, isOutput=False)
    al_d = nc.declare_dram_parameter("alpha", [BL, T], F32, isOutput=False)
    u_d = nc.declare_dram_parameter("u", [BL, 1], F32, isOutput=False)
    mW_d = nc.declare_dram_parameter("memory_W", [AD, E], BF16, isOutput=False)  # x64
    qW_d = nc.declare_dram_parameter("query_W", [AD, RNN], BF16, isOutput=False)
    vW_d = nc.declare_dram_parameter("v_W", [1, AD], F32, isOutput=False)
    cW_d = nc.declare_dram_parameter("loc_conv_W", [NF, 2, K], F32, isOutput=False)
    dW_d = nc.declare_dram_parameter("loc_dense_W", [AD, NF], F32, isOutput=False)
    taW_d = nc.declare_dram_parameter("ta_W", [1, E + RNN], F32, isOutput=False)
    tab_d = nc.declare_dram_parameter("ta_b", [1, 1], F32, isOutput=False)
    smW_d = nc.declare_dram_parameter("self_memory_W", [AD, EL], BF16, isOutput=False)
    sqW_d = nc.declare_dram_parameter("self_query_W", [AD, RNN], BF16, isOutput=False)
    svW_d = nc.declare_dram_parameter("self_v_W", [1, AD], F32, isOutput=False)
    out_d = nc.declare_dram_parameter("out", [BL, OUT_W], F32, isOutput=True)

    with ExitStack() as ctx:
        tc = ctx.enter_context(TileContext(nc))
        cpool = ctx.enter_context(tc.tile_pool(name="const", bufs=1))
        mpool = ctx.enter_context(tc.tile_pool(name="mem", bufs=8))
        tpool = ctx.enter_context(tc.tile_pool(name="tanhp", bufs=3))
        cspool = ctx.enter_context(tc.tile_pool(name="convsp", bufs=2))
        bsbpool = ctx.enter_context(tc.tile_pool(name="bsb", bufs=2))
        # separate STT scratch pools per engine — a shared pool would make
        # every row's scratch reuse the other engine's slots, serializing the
        # DVE and Pool ctx streams into lockstep
        scpool_v = ctx.enter_context(tc.tile_pool(name="scrv", bufs=2))
        scpool_p = ctx.enter_context(tc.tile_pool(name="scrp", bufs=2))
        scpool_a = ctx.enter_context(tc.tile_pool(name="scra", bufs=2))
        rtp = ctx.enter_context(tc.tile_pool(name="rtp", bufs=1))
        ldp = ctx.enter_context(tc.tile_pool(name="ldp", bufs=2))
        # PSUM: 8 banks of [128, 512] f32. pm/conv 2 + bc 2x2 + e-segs 2.
        ppm = ctx.enter_context(tc.tile_pool(name="ppm", bufs=2, space="PSUM"))
        pbc = ctx.enter_context(tc.tile_pool(name="pbc", bufs=2, space="PSUM"))
        ppe = ctx.enter_context(tc.tile_pool(name="ppe", bufs=1, space="PSUM"))

        # identity goes FIRST on the gpsimd queue — everything transposes
        # through it, and gpsimd also carries the phase-2 DMA stream
        ident = cpool.tile([128, 128], F32, tag="ident")
        make_identity(nc, ident[:])
        ident_bf = cpool.tile([128, 128], BF16, tag="ident_bf")
        nc.scalar.activation(ident_bf[:], ident[:], AF.Copy)

        # ---------------- DMA issue: weights + phase-1 memory on sync,
        # ---------------- im2col + phase-2 memory on gpsimd ----------------
        # Memory rides in the "(p c) t" interleaved layout: partition p holds
        # E rows 4p..4p+3 as one contiguous 4000B run per row, so one DMA can
        # carry TWO batch rows in a 3-dim access pattern (the per-DMA fixed
        # cost - dge + semaphore - was pacing each queue to one 0.5MB tile
        # per ~3us).  E row 4p+c lives at col (b%2)*4T + c*T.
        def load_pair(mem_dram, pair, eng):
            memT = mpool.tile([128, 8 * T], F8, tag="memT", name="memT")
            src = _dc.replace(
                mem_dram[:],
                ap=[[4 * T, 128], [E * T, 2], [1, 4 * T]],
                offset=pair * 2 * E * T,
            )
            eng.dma_start(
                out=memT[:].rearrange("p (b x) -> p b x", b=2),
                in_=src,
            )
            return memT

        def pair_view(memTs, b):
            return memTs[b // 2], (b % 2) * 4 * T

        mW_nat = ldp.tile([AD, E], BF16, tag="mw_nat", name="mW_nat")
        nc.sync.dma_start(out=mW_nat[:], in_=mW_d[:])
        memTs_p1 = {0: load_pair(mem_d, 0, nc.sync)}
        qW_nat = ldp.tile([AD, RNN], BF16, tag="qw_nat", name="qW_nat")
        nc.sync.dma_start(out=qW_nat[:], in_=qW_d[:])
        q_sb = cpool.tile([BL, RNN], F32, tag="q_sb")
        nc.sync.dma_start(out=q_sb[:], in_=q_d[:])
        memTs_p1[1] = load_pair(mem_d, 1, nc.sync)
        convWT_f = cpool.tile([2 * K, NF], F32, tag="convWT_f")  # [(c k), o]
        nc.sync.dma_start(out=convWT_f[:], in_=cW_d[:].rearrange("o c k -> (c k) o"))
        ldWT_f = cpool.tile([NF, AD], F32, tag="ldWT_f")
        nc.sync.dma_start(out=ldWT_f[:], in_=dW_d[:].rearrange("a f -> f a"))
        v_sb = cpool.tile([128, 2], F32, tag="v_sb")
        nc.sync.dma_start(out=v_sb[:, 0:1], in_=vW_d[:].rearrange("o a -> a o"))
        nc.sync.dma_start(out=v_sb[:, 1:2], in_=svW_d[:].rearrange("o a -> a o"))
        memTs_p1[2] = load_pair(mem_d, 2, nc.sync)

        # per-row scalars: 0=u 1=1-u 2..3 alpha-sums 4=s_ta1 5=ta_b 6..7 sig-sums
        cols = cpool.tile([BL, 12], F32, tag="cols")
        nc.sync.dma_start(
            out=cols[:, 5:6], in_=_dc.replace(tab_d[:], ap=[[0, BL], [1, 1]])
        )
        awc_rt = rtp.tile([BL, T], F32, tag="awc_rt", name="awc_rt")
        nc.sync.dma_start(out=awc_rt[:], in_=awc_d[:])
        memTs_p1[3] = load_pair(mem_d, 3, nc.sync)

        smW_nat = ldp.tile([AD, EL], BF16, tag="smw_nat", name="smW_nat")
        nc.sync.dma_start(out=smW_nat[:], in_=smW_d[:])
        sqW_nat = ldp.tile([AD, RNN], BF16, tag="sqw_nat", name="sqW_nat")
        nc.sync.dma_start(out=sqW_nat[:], in_=sqW_d[:])
        taWb = cpool.tile([BL, E + RNN], F32, tag="taWb")
        nc.sync.dma_start(
            out=taWb[:], in_=_dc.replace(taW_d[:], ap=[[0, BL], [1, E + RNN]])
        )
        alpha_rt = rtp.tile([BL, T], F32, tag="alpha_rt", name="alpha_rt")
        nc.sync.dma_start(out=alpha_rt[:], in_=al_d[:])
        nc.sync.dma_start(out=cols[:, 0:1], in_=u_d[:])

        # im2col of [attention_weights; attention_weights_cum] for the
        # location conv: rows (c, k), cols (b, t); zero-padded edges via a
        # DRAM bounce so the sliding window is 2 DMAs.  All on gpsimd.
        TP = T + 2 * PAD
        pad_d = nc.dram_tensor("awc_pad", [2 * BL, TP], BF16)
        zero_sb = cpool.tile([2 * BL, 16], BF16, tag="zero_sb")
        nc.vector.memset(zero_sb[:], 0.0)
        nc.gpsimd.dma_start(out=pad_d[:, 0:PAD], in_=zero_sb[:, 0:PAD])
        nc.gpsimd.dma_start(out=pad_d[:, TP - PAD : TP], in_=zero_sb[:, 0:PAD])
        nc.gpsimd.dma_start(out=pad_d[0:BL, PAD : PAD + T], in_=aw_d[:])
        nc.gpsimd.dma_start(out=pad_d[BL : 2 * BL, PAD : PAD + T], in_=awcb_d[:])
        im2 = cpool.tile([2 * K, BL * T], BF16, tag="im2")
        for c in range(2):
            src = _dc.replace(
                pad_d[:], ap=[[1, K], [TP, BL], [1, T]], offset=c * BL * TP
            )
            nc.gpsimd.dma_start(
                out=im2[c * K : (c + 1) * K, :].rearrange("k (b t) -> k b t", b=BL),
                in_=src,
            )
        memTs_p2 = {0: load_pair(smem_d, 0, nc.gpsimd)}

        # sel[:, b*128:(b+1)*128] is an [8, 128] selector whose row b is all
        # ones: bc = sel_b.T @ w_bf broadcasts w row b across 128 partitions.
        # Built with two affine selects on the gpsimd queue, slotted after
        # smemT1 so neither the conv path nor the bc matmuls wait on it:
        # keep 1.0 where 0 <= j - 128*p <= 127, else fill 0.
        sel = cpool.tile([BL, BL * 128], BF16, tag="sel")
        nc.gpsimd.memset(sel[:], 1.0)
        nc.gpsimd.affine_select(
            out=sel[:],
            in_=sel[:],
            compare_op=ALU.is_ge,
            fill=0.0,
            base=0,
            pattern=[[1, BL * 128]],
            channel_multiplier=-128,
        )
        nc.gpsimd.affine_select(
            out=sel[:],
            in_=sel[:],
            compare_op=ALU.is_ge,
            fill=0.0,
            base=127,
            pattern=[[-1, BL * 128]],
            channel_multiplier=128,
        )

        for pr in range(1, BL // 2):
            memTs_p2[pr] = load_pair(smem_d, pr, nc.gpsimd)

        # ---------------- constants / on-chip weight prep ----------------
        # setup copies run on DVE (idle until the phase-1 chain) so the ACT
        # queue reaches conv_s/tanh — and thus sigmoid1 — as early as possible
        def transpose_into(dst, nat, ncols, dt, eng_copy, interleave=False):
            idn = {F32: ident, BF16: ident_bf}[dt]
            for c in range(ncols // 128):
                if interleave:
                    # column set {4j + c}: gives W.T rows for the "(p c) t"
                    # memory layout where E row 4p+c sits at partition p
                    sl = nat[:, c : c + 1]
                    src = _dc.replace(sl, ap=[sl.ap[0], [4, 128]])
                else:
                    src = nat[:, c * 128 : (c + 1) * 128]
                tp = ppm.tile([128, 512], dt, tag="pm", name="wtp")
                nc.tensor.transpose(tp[:, 0:128], src, idn[:])
                if eng_copy is nc.scalar:
                    nc.scalar.activation(
                        dst[:, c * 128 : (c + 1) * 128], tp[:, 0:128], AF.Copy
                    )
                else:
                    eng_copy.tensor_scalar_add(
                        dst[:, c * 128 : (c + 1) * 128], tp[:, 0:128], 0.0
                    )

        mWT = cpool.tile([128, E], F8, tag="mWT")
        transpose_into(mWT, mW_nat, E, BF16, nc.scalar, interleave=True)
        qWT = cpool.tile([128, RNN], BF16, tag="qWT")
        transpose_into(qWT, qW_nat, RNN, BF16, nc.vector)
        # smWT/sqWT/spq are deferred into the phase loop (issued after the
        # phase-1 rows) so their transposes don't delay the first pm

        qT = cpool.tile([128, 8 * BL], BF16, tag="qT")  # cols (rchunk, b)
        for c in range(8):
            tp = ppm.tile([128, 512], F32, tag="pm", name="qtp")
            nc.tensor.transpose(
                tp[:, 0:BL], q_sb[:, c * 128 : (c + 1) * 128], ident[0:BL, 0:BL]
            )
            nc.vector.tensor_scalar_add(qT[:, c * BL : (c + 1) * BL], tp[:, 0:BL], 0.0)

        convWT = cpool.tile([2 * K, NF], BF16, tag="convWT")
        nc.vector.tensor_scalar_add(convWT[:], convWT_f[:], 0.0)
        ldWT = cpool.tile([NF, AD], BF16, tag="ldWT")  # x64 to match fp8 weight scale
        nc.vector.tensor_scalar_mul(ldWT[:], ldWT_f[:], WS)

        # vmat[:, bi*BL + j] = v if j == bi else 0 — row-masked v so the
        # e contraction for row bi lands in PSUM partition bi.
        def masked_v(col, name):
            t = cpool.tile([128, BL * BL], BF16, tag=name)
            nc.vector.memset(t[:], 0.0)
            for bi in range(BL):
                c = bi * BL + bi
                nc.vector.tensor_scalar_add(
                    t[:, c : c + 1], v_sb[:, col : col + 1], 0.0
                )
            return t

        vmat = masked_v(0, "vmat")
        svmat = masked_v(1, "svmat")

        # context / u_new staging for the whole local batch (partitions 0..BL)
        out_tile = cpool.tile([BL, OUT_W], F32, tag="out_tile")
        scr8 = out_tile[0:BL, CUM0 : CUM0 + RNN]  # never reaches DRAM from here

        # ---------------- query projections (pq, spq) ----------------
        def project_query(wT, name):
            ps = ppm.tile([128, 512], F32, tag="pm", name="pq_ps")
            for rc in range(8):
                nc.tensor.matmul(
                    ps[:, 0:BL],
                    lhsT=wT[:, rc * 128 : (rc + 1) * 128],
                    rhs=qT[:, rc * BL : (rc + 1) * BL],
                    start=(rc == 0),
                    stop=(rc == 7),
                )
            sb = cpool.tile([128, BL], F32, tag=name)
            nc.vector.tensor_scalar_add(sb[:], ps[:, 0:BL], 0.0)
            return sb

        pq_sb = project_query(qWT, "pq_sb")
        smWT = sqWT = spq_sb = None  # created after the first ph1 row issues

        # ---------------- per-phase pieces ----------------
        def row_matmuls(memTs, wT, pq, v, with_loc, e_segs, b):
            """conv (optional) + pm (fp8 DoubleRow) + tanh + e-matmul for row b."""
            memT, roff = pair_view(memTs, b)
            wT3 = wT[:].rearrange("p (g m) -> p g m", g=4)  # 4 k-tiles of 128
            if with_loc:
                cps = ppm.tile([128, 512], F32, tag="pm", name="cps")
                conv_s = cspool.tile([NF, T], BF16, tag="convs", name="conv_s")
                for si, (t0, tl) in enumerate(SEGS):
                    nc.tensor.matmul(
                        cps[0:NF, 0:tl],
                        lhsT=convWT[:],
                        rhs=im2[:, b * T + t0 : b * T + t0 + tl],
                        start=True,
                        stop=True,
                        skip_group_check=True,
                    )
                    # split the psum->sbuf bounce across ACT and DVE so
                    # neither paces the row stream
                    if si == 0:
                        nc.scalar.activation(
                            conv_s[:, t0 : t0 + tl], cps[0:NF, 0:tl], AF.Copy
                        )
                    else:
                        nc.vector.tensor_scalar_add(
                            conv_s[:, t0 : t0 + tl], cps[0:NF, 0:tl], 0.0
                        )
            th = tpool.tile([128, T], BF16, tag="tanh", name="th")
            for si, (t0, tl) in enumerate(SEGS):
                pm = ppm.tile([128, 512], F32, tag="pm", name="pm")
                for g in range(2):  # two DoubleRow matmuls cover 4 k-tiles
                    sl = memT[:, roff + 2 * g * T + t0 : roff + 2 * g * T + t0 + tl]
                    rhs = _dc.replace(sl, ap=[sl.ap[0], [T, 2], [1, tl]])
                    nc.tensor.matmul(
                        pm[:, 0:tl],
                        lhsT=wT3[:, 2 * g : 2 * g + 2, :],
                        rhs=rhs,
                        start=(g == 0),
                        stop=(g == 1 and not with_loc),
                        perf_mode=PM_DR,
                        skip_group_check=True,
                    )
                if with_loc:
                    nc.tensor.matmul(
                        pm[:, 0:tl],
                        lhsT=ldWT[:],
                        rhs=conv_s[:, t0 : t0 + tl],
                        start=False,
                        stop=True,
                        skip_group_check=True,
                    )
                nc.scalar.activation(
                    th[:, t0 : t0 + tl],
                    pm[:, 0:tl],
                    AF.Tanh,
                    bias=pq[:, b : b + 1],
                    scale=1.0 / WS,
                )
                nc.tensor.matmul(
                    e_segs[si][0:BL, 0:tl],
                    lhsT=v[:, b * BL : (b + 1) * BL],
                    rhs=th[:, t0 : t0 + tl],
                    start=(b == 0),
                    stop=(b == BL - 1),
                    skip_group_check=True,
                )

        def make_esegs(name):
            return [
                ppe.tile([BL, 512], F32, tag=f"pe{si}", name=f"{name}{si}")
                for si in range(2)
            ]

        def phase_chain(e_segs, with_loc, w_rt, w_bf, sig_rt):
            """sigmoid + normalization (+ alpha recurrence for phase 1)."""
            for si, (t0, tl) in enumerate(SEGS):
                nc.scalar.activation(
                    sig_rt[:, t0 : t0 + tl],
                    e_segs[si][0:BL, 0:tl],
                    AF.Sigmoid,
                    accum_out=cols[:, 6 + si : 7 + si],
                )
            if with_loc:
                nc.vector.tensor_add(cols[:, 8:9], cols[:, 6:7], cols[:, 7:8])
                nc.vector.reciprocal(cols[:, 9:10], cols[:, 8:9])
                # cum_new = awc + sig/sum(sig) in one fused op
                anew_rt = rtp.tile([BL, T], F32, tag="anew_rt", name="anew_rt")
                nc.vector.scalar_tensor_tensor(
                    out=anew_rt[:],
                    in0=sig_rt[:],
                    scalar=cols[:, 9:10],
                    in1=awc_rt[:],
                    op0=ALU.mult,
                    op1=ALU.add,
                )
                nc.sync.dma_start(out=out_d[:, CUM0 : CUM0 + T], in_=anew_rt[:])
                # monotonic alpha recurrence; the sigmoid-normalizing scalar
                # cancels in alignments = x/sum(x), so run the chain off raw
                # sig:  base = (1-u)*alpha + u*shift(alpha)
                #       w    = (base + 1e-8)*sig, normalized
                nc.vector.tensor_scalar(
                    out=cols[:, 1:2],
                    in0=cols[:, 0:1],
                    scalar1=-1.0,
                    scalar2=1.0,
                    op0=ALU.mult,
                    op1=ALU.add,
                )
                shift_rt = rtp.tile([BL, T], F32, tag="shift_rt", name="shift_rt")
                base_rt = rtp.tile([BL, T], F32, tag="base_rt", name="base_rt")
                nc.vector.memset(shift_rt[:, 0:1], 0.0)
                nc.vector.tensor_scalar_mul(
                    shift_rt[:, 1:T], alpha_rt[:, 0 : T - 1], cols[:, 0:1]
                )
                nc.vector.scalar_tensor_tensor(
                    out=base_rt[:],
                    in0=alpha_rt[:],
                    scalar=cols[:, 1:2],
                    in1=shift_rt[:],
                    op0=ALU.mult,
                    op1=ALU.add,
                )
                nc.vector.scalar_tensor_tensor(
                    out=base_rt[:],
                    in0=base_rt[:],
                    scalar=1e-8,
                    in1=sig_rt[:],
                    op0=ALU.add,
                    op1=ALU.mult,
                    accum_out=cols[:, 2:3],
                )
                nc.vector.reciprocal(cols[:, 3:4], cols[:, 2:3])
                nc.vector.tensor_scalar_mul(w_rt[:], base_rt[:], cols[:, 3:4])
            else:
                nc.vector.tensor_add(cols[:, 8:9], cols[:, 6:7], cols[:, 7:8])
                nc.vector.reciprocal(cols[:, 9:10], cols[:, 8:9])
                nc.vector.tensor_scalar_mul(w_rt[:], sig_rt[:], cols[:, 9:10])
            nc.scalar.activation(w_bf[:], w_rt[:], AF.Copy)

        def row_ctx(memTs, w_bf, ctxT, b):
            """bc broadcast matmul + 4 full-T weighted-sum accumulations.

            DVE rows: scalar_tensor_tensor with f32 accum (verified 2e-6).
            Pool rows: gpsimd tensor_tensor f8 x bf16 -> f32 product, then
            ACT Copy with accum_out (f32 accumulation follows the f32 input;
            gpsimd STT is rejected by codegen and ACT accumulation over a
            bf16 input is only ~1e-1 accurate).
            bc is bounced to bf16 SBUF for everyone: gpsimd cannot read
            PSUM, and the f8 x f32 STT path accumulates poorly.
            """
            memT, roff = pair_view(memTs, b)
            bc = pbc.tile([128, 1024], F32, tag="bc", name="bc")
            for t0, tl in SEGS:
                nc.tensor.matmul(
                    bc[:, t0 : t0 + tl],
                    lhsT=sel[:, b * 128 : (b + 1) * 128],
                    rhs=w_bf[:, t0 : t0 + tl],
                    start=True,
                    stop=True,
                    skip_group_check=True,
                )
            bc_sb = bsbpool.tile([128, T], BF16, tag="bcsb", name="bc_sb")
            nc.scalar.activation(bc_sb[:], bc[:, 0:T], AF.Copy)
            for c in range(4):
                cc = c * BL + b
                if b in DVE_ROWS:
                    scr = scpool_v.tile([128, T], BF16, tag="scr", name="scr")
                    nc.vector.scalar_tensor_tensor(
                        out=scr[:],
                        in0=memT[:, roff + c * T : roff + (c + 1) * T],
                        scalar=1.0,
                        in1=bc_sb[:],
                        op0=ALU.mult,
                        op1=ALU.mult,
                        accum_out=ctxT[:, cc : cc + 1],
                    )
                else:
                    scr32 = scpool_p.tile([128, T], F32, tag="scr32", name="scr32")
                    nc.gpsimd.tensor_tensor(
                        scr32[:],
                        memT[:, roff + c * T : roff + (c + 1) * T],
                        bc_sb[:],
                        ALU.mult,
                    )
                    dump = scpool_a.tile([128, T], BF16, tag="dump", name="dump")
                    nc.scalar.activation(
                        dump[:], scr32[:], AF.Copy, accum_out=ctxT[:, cc : cc + 1]
                    )

        def ctx_finalize(ctxT, ctx_off):
            # ctxT col (c, b) holds ctx[b, e] for e = 4p + c (interleaved
            # memory layout) -> transpose chunk c lands in out cols c::4
            for c in range(4):
                tp = ppm.tile([128, 512], F32, tag="pm", name="ctp")
                nc.tensor.transpose(
                    tp[0:BL, 0:128], ctxT[:, c * BL : (c + 1) * BL], ident[:]
                )
                sl = out_tile[0:BL, ctx_off + c : ctx_off + c + 1]
                dst = _dc.replace(sl, ap=[sl.ap[0], [4, 128]])
                nc.scalar.activation(dst, tp[0:BL, 0:128], AF.Copy)

        for _rep in range(repeat):
            if _rep > 0:
                memTs_p1 = {p: load_pair(mem_d, p, nc.sync) for p in range(BL // 2)}
                memTs_p2 = {p: load_pair(smem_d, p, nc.gpsimd) for p in range(BL // 2)}

            # ---- phase 1: location-sensitive monotonic attention ----
            e1 = make_esegs("e1")
            for b in range(BL):
                row_matmuls(memTs_p1, mWT, pq_sb, vmat, True, e1, b)
                if b == 1 and smWT is None:
                    # slot the phase-2 weight prep into the row stream's PE
                    # slack (rows are ACT/DVE-paced)
                    smWT = cpool.tile([128, EL], F8, tag="smWT")
                    transpose_into(smWT, smW_nat, EL, BF16, nc.scalar, interleave=True)
                    sqWT = cpool.tile([128, RNN], BF16, tag="sqWT")
                    transpose_into(sqWT, sqW_nat, RNN, BF16, nc.vector)
                    spq_sb = project_query(sqWT, "spq_sb")
            sig1 = rtp.tile([BL, T], F32, tag="sig1", name="sig1")
            w1 = rtp.tile([BL, T], F32, tag="w1", name="w1")
            w1_bf = rtp.tile([BL, T], BF16, tag="w1bf", name="w1_bf")
            phase_chain(e1, True, w1, w1_bf, sig1)
            nc.sync.dma_start(out=out_d[:, ALIGN0 : ALIGN0 + T], in_=w1[:])

            ctxT1 = cpool.tile([128, 4 * BL], F32, tag="ctxT1")
            e2 = make_esegs("e2")
            # interleave phase-2 row matmuls with phase-1 ctx so the PE queue
            # never parks behind a bc matmul waiting on DVE/Pool
            for b in range(BL):
                row_matmuls(memTs_p2, smWT, spq_sb, svmat, False, e2, b)
                row_ctx(memTs_p1, w1_bf, ctxT1, b)
            ctx_finalize(ctxT1, CTX0)

            # u_new = sigmoid([context, query] @ ta_W.T + ta_b)
            nc.vector.scalar_tensor_tensor(
                out=scr8[:, 0:E],
                in0=out_tile[0:BL, CTX0 : CTX0 + E],
                scalar=1.0,
                in1=taWb[:, 0:E],
                op0=ALU.mult,
                op1=ALU.mult,
                accum_out=cols[:, 4:5],
            )
            nc.vector.scalar_tensor_tensor(
                out=scr8[:, 0:RNN],
                in0=q_sb[:],
                scalar=1.0,
                in1=taWb[:, E : E + RNN],
                op0=ALU.mult,
                op1=ALU.mult,
                accum_out=cols[:, 10:11],
            )
            nc.vector.tensor_add(cols[:, 11:12], cols[:, 4:5], cols[:, 10:11])
            nc.scalar.activation(
                out_tile[0:BL, UN0 : UN0 + 1],
                cols[:, 11:12],
                AF.Sigmoid,
                bias=cols[:, 5:6],
            )

            # ---- phase 2: additive self-attention ----
            sig2 = rtp.tile([BL, T], F32, tag="sig2", name="sig2")
            w2 = rtp.tile([BL, T], F32, tag="w2", name="w2")
            w2_bf = rtp.tile([BL, T], BF16, tag="w2bf", name="w2_bf")
            phase_chain(e2, False, w2, w2_bf, sig2)
            nc.sync.dma_start(out=out_d[:, W2_0 : W2_0 + T], in_=w2[:])

            ctxT2 = cpool.tile([128, 4 * BL], F32, tag="ctxT2")
            for b in range(BL):
                row_ctx(memTs_p2, w2_bf, ctxT2, b)
            ctx_finalize(ctxT2, CTX2_0)

            nc.sync.dma_start(
                out=out_d[:, CTX0 : CTX0 + E], in_=out_tile[:, CTX0 : CTX0 + E]
            )
            nc.sync.dma_start(
                out=out_d[:, UN0 : UN0 + 1], in_=out_tile[:, UN0 : UN0 + 1]
            )
            nc.sync.dma_start(
                out=out_d[:, CTX2_0 : CTX2_0 + EL],
                in_=out_tile[:, CTX2_0 : CTX2_0 + EL],
            )

    if finalize:
        nc.finalize()
    return nc


_NC = None
RUN_KWARGS: dict = {}   # test harness can set {"trace": True}
LAST_RESULT = None      # BassKernelResults of the most recent kernel() call


def _get_nc():
    global _NC
    if _NC is None:
        _NC = build_nc()
    return _NC


def make_in_map(shard: dict) -> dict:
    """Device in_map for ONE core's shard (keys as in setup_inputs)."""
    f = lambda k: np.ascontiguousarray(np.asarray(shard[k], dtype=np.float32))
    bf = ml_dtypes.bfloat16
    f8 = ml_dtypes.float8_e4m3
    return {
        "query": f("query"),
        "memory": np.ascontiguousarray(f("memory").transpose(0, 2, 1).astype(f8)),
        "self_memory": np.ascontiguousarray(
            f("self_memory").transpose(0, 2, 1).astype(f8)
        ),
        "attention_weights": f("attention_weights").astype(bf),
        "attention_weights_cum": f("attention_weights_cum"),
        "awc_bf": f("attention_weights_cum").astype(bf),
        "alpha": f("alpha"),
        "u": f("u"),
        "memory_W": (f("memory_W") * WS).astype(bf),
        "query_W": f("query_W").astype(bf),
        "v_W": f("v_W"),
        "loc_conv_W": f("loc_conv_W"),
        "loc_dense_W": f("loc_dense_W"),
        "ta_W": f("ta_W"),
        "ta_b": f("ta_b").reshape(1, 1),
        "self_memory_W": (f("self_memory_W") * WS).astype(bf),
        "self_query_W": f("self_query_W").astype(bf),
        "self_v_W": f("self_v_W"),
    }


def kernel(**inputs) -> np.ndarray:
    f = lambda k: np.ascontiguousarray(np.asarray(inputs[k], dtype=np.float32))
    bf = ml_dtypes.bfloat16
    f8 = ml_dtypes.float8_e4m3
    rep = {
        "memory_W": (f("memory_W") * WS).astype(bf),
        "query_W": f("query_W").astype(bf),
        "v_W": f("v_W"),
        "loc_conv_W": f("loc_conv_W"),
        "loc_dense_W": f("loc_dense_W"),
        "ta_W": f("ta_W"),
        "ta_b": f("ta_b").reshape(1, 1),
        "self_memory_W": (f("self_memory_W") * WS).astype(bf),
        "self_query_W": f("self_query_W").astype(bf),
        "self_v_W": f("self_v_W"),
    }
    mem_t = np.ascontiguousarray(f("memory").transpose(0, 2, 1).astype(f8))
    smem_t = np.ascontiguousarray(f("self_memory").transpose(0, 2, 1).astype(f8))
    aw_bf = f("attention_weights").astype(bf)
    awc = f("attention_weights_cum")
    awc_bf = awc.astype(bf)
    q = f("query")
    alpha = f("alpha")
    u = f("u")
    in_maps = []
    for i in range(NCORES):
        sl = slice(i * BL, (i + 1) * BL)
        m = dict(rep)
        m["query"] = q[sl]
        m["memory"] = mem_t[sl]
        m["self_memory"] = smem_t[sl]
        m["attention_weights"] = aw_bf[sl]
        m["attention_weights_cum"] = awc[sl]
        m["awc_bf"] = awc_bf[sl]
        m["alpha"] = alpha[sl]
        m["u"] = u[sl]
        in_maps.append(m)
    global LAST_RESULT
    res = run_bass_kernel_spmd(
        _get_nc(), in_maps, core_ids=list(range(NCORES)), **RUN_KWARGS
    )
    LAST_RESULT = res
    return np.concatenate([res.results[i]["out"] for i in range(NCORES)], axis=0)


# revision 38
# speedup vs baseline: 2.7588x; 1.4500x over previous
"""Trainium2 Bass kernel for nn_Attention_15410342658774 (v2).

Location-sensitive monotonic attention + additive self-attention
(Tacotron-style), B=64, T=1000, E=EL=512, RNN=1024, AD=128.

Pure data parallel across 8 NeuronCores (8 batch rows each, weights
replicated).  Host pre-transposes `memory`/`self_memory` to [B, E, T]
and casts to fp8e4 (halves the HBM stream vs bf16 and enables DoubleRow
matmuls); weight matrices ride along as fp8e4 scaled by 64 (folded back
out via the tanh activation's input scale).

v2 changes vs the 162us baseline (cost-model-driven):
  - fp8e4 memory stream: DMA_ENGINES floor 56us -> ~28us; DoubleRow
    fp8 matmuls for pm (2 k-tiles per instruction at 0.5 cyc/row).
  - All big DMAs on dedicated queues (sync for phase 1 + weights,
    gpsimd for phase 2 + im2col) - never on the ACT/DVE/PE queues,
    since a dma_start occupies its queue for the whole transfer.
  - ctx accumulation (the 82us DVE hotspot; scalar_tensor_tensor has
    no DVE fast mode) split DVE/Pool: DVE rows read the bc broadcast
    straight from PSUM, Pool rows get an ACT-copied bf16 SBUF view
    (gpsimd cannot touch PSUM).
  - bc broadcast matmuls in bf16 (they were f32 = 4 cyc/row on PE).
  - reductions fused into producers via accum_out (sigmoid-sum,
    alpha-chain sum).
  - PSUM budget reworked to exactly 8 banks: pm 2 + bc 2x2 + e-segs 2.

Hardware constraints baked in (from the v1 session + cost model):
  - matmul operands/outputs must start at partition 0/32/64; PSUM
    matmul outputs must not cross a 2KB bank.
  - DMA access patterns: at most 3 [step,count] dims, innermost step 1.
  - TENSOR_TENSOR_REDUCE crashes the exec unit; scalar_tensor_tensor's
    accum_out is the working per-partition reduction.
  - gpsimd (Pool) engine: SBUF only, no PSUM access.
"""

import dataclasses as _dc
import sys

import numpy as np

_TRN = "/opt/trn_rl_repo"
if _TRN not in sys.path:
    sys.path.insert(0, _TRN)

from contextlib import ExitStack

import ml_dtypes

import concourse.bacc as bacc
import concourse.bass as bass
import concourse.mybir as mybir
from concourse.bass_utils import run_bass_kernel_spmd
from concourse.masks import make_identity
from concourse.tile import TileContext

B, T = 64, 1000
E, EL, RNN, AD = 512, 512, 1024, 128
NF, K = 32, 31
PAD = (K - 1) // 2
NCORES = 8
BL = B // NCORES  # 8 batch rows per core
F32 = mybir.dt.float32
BF16 = mybir.dt.bfloat16
F8 = mybir.dt.float8e4
AF = mybir.ActivationFunctionType
ALU = mybir.AluOpType
AX = mybir.AxisListType
PM_DR = mybir.MatmulPerfMode.DoubleRow
SEGS = [(0, 512), (512, 488)]  # T split at the 512-float PSUM bank boundary
WS = 64.0  # fp8 weight pre-scale (exact power of two)

# output packing: [context(E) | alignments(T) | u_new(1) | cum_new(T) | ctx2(EL) | w2(T)]
CTX0 = 0
ALIGN0 = E
UN0 = E + T
CUM0 = E + T + 1
CTX2_0 = E + 2 * T + 1
W2_0 = E + EL + 2 * T + 1
OUT_W = E + EL + 3 * T + 1  # 4025

DVE_ROWS = (0, 1, 2, 3, 4, 5, 6, 7)  # all ctx rows on DVE (Pool TT was slow on HW)


def build_nc(finalize: bool = True, repeat: int = 1) -> bass.Bass:
    nc = bacc.Bacc()

    q_d = nc.declare_dram_parameter("query", [BL, RNN], F32, isOutput=False)
    # pre-transposed [E, T] per row, fp8e4 (host-prepared)
    mem_d = nc.declare_dram_parameter("memory", [BL, E, T], F8, isOutput=False)
    smem_d = nc.declare_dram_parameter("self_memory", [BL, EL, T], F8, isOutput=False)
    aw_d = nc.declare_dram_parameter("attention_weights", [BL, T], BF16, isOutput=False)
    awc_d = nc.declare_dram_parameter(
        "attention_weights_cum", [BL, T], F32, isOutput=False
    )
    awcb_d = nc.declare_dram_parameter("awc_bf", [BL, T], BF16, isOutput=False)
    al_d = nc.declare_dram_parameter("alpha", [BL, T], F32, isOutput=False)
    u_d = nc.declare_dram_parameter("u", [BL, 1], F32, isOutput=False)
    mW_d = nc.declare_dram_parameter("memory_W", [AD, E], BF16, isOutput=False)  # x64
    qW_d = nc.declare_dram_parameter("query_W", [AD, RNN], BF16, isOutput=False)
    vW_d = nc.declare_dram_parameter("v_W", [1, AD], F32, isOutput=False)
    cW_d = nc.declare_dram_parameter("loc_conv_W", [NF, 2, K], F32, isOutput=False)
    dW_d = nc.declare_dram_parameter("loc_dense_W", [AD, NF], F32, isOutput=False)
    taW_d = nc.declare_dram_parameter("ta_W", [1, E + RNN], F32, isOutput=False)
    tab_d = nc.declare_dram_parameter("ta_b", [1, 1], F32, isOutput=False)
    smW_d = nc.declare_dram_parameter("self_memory_W", [AD, EL], BF16, isOutput=False)
    sqW_d = nc.declare_dram_parameter("self_query_W", [AD, RNN], BF16, isOutput=False)
    svW_d = nc.declare_dram_parameter("self_v_W", [1, AD], F32, isOutput=False)
    out_d = nc.declare_dram_parameter("out", [BL, OUT_W], F32, isOutput=True)

    with ExitStack() as ctx:
        tc = ctx.enter_context(TileContext(nc))
        cpool = ctx.enter_context(tc.tile_pool(name="const", bufs=1))
        mpool = ctx.enter_context(tc.tile_pool(name="mem", bufs=8))
        tpool = ctx.enter_context(tc.tile_pool(name="tanhp", bufs=3))
        cspool = ctx.enter_context(tc.tile_pool(name="convsp", bufs=2))
        bsbpool = ctx.enter_context(tc.tile_pool(name="bsb", bufs=2))
        # separate STT scratch pools per engine — a shared pool would make
        # every row's scratch reuse the other engine's slots, serializing the
        # DVE and Pool ctx streams into lockstep
        scpool_v = ctx.enter_context(tc.tile_pool(name="scrv", bufs=2))
        scpool_p = ctx.enter_context(tc.tile_pool(name="scrp", bufs=2))
        scpool_a = ctx.enter_context(tc.tile_pool(name="scra", bufs=2))
        rtp = ctx.enter_context(tc.tile_pool(name="rtp", bufs=1))
        ldp = ctx.enter_context(tc.tile_pool(name="ldp", bufs=2))
        # PSUM: 8 banks of [128, 512] f32. pm/conv 2 + bc 2x2 + e-segs 2.
        ppm = ctx.enter_context(tc.tile_pool(name="ppm", bufs=2, space="PSUM"))
        pbc = ctx.enter_context(tc.tile_pool(name="pbc", bufs=2, space="PSUM"))
        ppe = ctx.enter_context(tc.tile_pool(name="ppe", bufs=1, space="PSUM"))

        # identity goes FIRST on the gpsimd queue — everything transposes
        # through it, and gpsimd also carries the phase-2 DMA stream
        ident = cpool.tile([128, 128], F32, tag="ident")
        make_identity(nc, ident[:])
        ident_bf = cpool.tile([128, 128], BF16, tag="ident_bf")
        nc.scalar.activation(ident_bf[:], ident[:], AF.Copy)

        # ---------------- DMA issue: weights + phase-1 memory on sync,
        # ---------------- im2col + phase-2 memory on gpsimd ----------------
        # Memory rides in the "(p c) t" interleaved layout: partition p holds
        # E rows 4p..4p+3 as one contiguous 4000B run per row, so one DMA can
        # carry TWO batch rows in a 3-dim access pattern (the per-DMA fixed
        # cost - dge + semaphore - was pacing each queue to one 0.5MB tile
        # per ~3us).  E row 4p+c lives at col (b%2)*4T + c*T.
        def load_pair(mem_dram, pair, eng):
            memT = mpool.tile([128, 8 * T], F8, tag="memT", name="memT")
            src = _dc.replace(
                mem_dram[:],
                ap=[[4 * T, 128], [E * T, 2], [1, 4 * T]],
                offset=pair * 2 * E * T,
            )
            eng.dma_start(
                out=memT[:].rearrange("p (b x) -> p b x", b=2),
                in_=src,
            )
            return memT

        def pair_view(memTs, b):
            return memTs[b // 2], (b % 2) * 4 * T

        mW_nat = ldp.tile([AD, E], BF16, tag="mw_nat", name="mW_nat")
        nc.sync.dma_start(out=mW_nat[:], in_=mW_d[:])
        memTs_p1 = {0: load_pair(mem_d, 0, nc.sync)}
        qW_nat = ldp.tile([AD, RNN], BF16, tag="qw_nat", name="qW_nat")
        nc.sync.dma_start(out=qW_nat[:], in_=qW_d[:])
        q_sb = cpool.tile([BL, RNN], F32, tag="q_sb")
        nc.sync.dma_start(out=q_sb[:], in_=q_d[:])
        memTs_p1[1] = load_pair(mem_d, 1, nc.sync)
        convWT_f = cpool.tile([2 * K, NF], F32, tag="convWT_f")  # [(c k), o]
        nc.sync.dma_start(out=convWT_f[:], in_=cW_d[:].rearrange("o c k -> (c k) o"))
        ldWT_f = cpool.tile([NF, AD], F32, tag="ldWT_f")
        nc.sync.dma_start(out=ldWT_f[:], in_=dW_d[:].rearrange("a f -> f a"))
        v_sb = cpool.tile([128, 2], F32, tag="v_sb")
        nc.sync.dma_start(out=v_sb[:, 0:1], in_=vW_d[:].rearrange("o a -> a o"))
        nc.sync.dma_start(out=v_sb[:, 1:2], in_=svW_d[:].rearrange("o a -> a o"))
        memTs_p1[2] = load_pair(mem_d, 2, nc.sync)

        # per-row scalars: 0=u 1=1-u 2..3 alpha-sums 4=s_ta1 5=ta_b 6..7 sig-sums
        cols = cpool.tile([BL, 12], F32, tag="cols")
        nc.sync.dma_start(
            out=cols[:, 5:6], in_=_dc.replace(tab_d[:], ap=[[0, BL], [1, 1]])
        )
        awc_rt = rtp.tile([BL, T], F32, tag="awc_rt", name="awc_rt")
        nc.sync.dma_start(out=awc_rt[:], in_=awc_d[:])
        memTs_p1[3] = load_pair(mem_d, 3, nc.sync)

        smW_nat = ldp.tile([AD, EL], BF16, tag="smw_nat", name="smW_nat")
        nc.sync.dma_start(out=smW_nat[:], in_=smW_d[:])
        sqW_nat = ldp.tile([AD, RNN], BF16, tag="sqw_nat", name="sqW_nat")
        nc.sync.dma_start(out=sqW_nat[:], in_=sqW_d[:])
        taWb = cpool.tile([BL, E + RNN], F32, tag="taWb")
        nc.sync.dma_start(
            out=taWb[:], in_=_dc.replace(taW_d[:], ap=[[0, BL], [1, E + RNN]])
        )
        alpha_rt = rtp.tile([BL, T], F32, tag="alpha_rt", name="alpha_rt")
        nc.sync.dma_start(out=alpha_rt[:], in_=al_d[:])
        nc.sync.dma_start(out=cols[:, 0:1], in_=u_d[:])

        # im2col of [attention_weights; attention_weights_cum] for the
        # location conv: rows (c, k), cols (b, t); zero-padded edges via a
        # DRAM bounce so the sliding window is 2 DMAs.  All on gpsimd.
        TP = T + 2 * PAD
        pad_d = nc.dram_tensor("awc_pad", [2 * BL, TP], BF16)
        zero_sb = cpool.tile([2 * BL, 16], BF16, tag="zero_sb")
        nc.vector.memset(zero_sb[:], 0.0)
        nc.gpsimd.dma_start(out=pad_d[:, 0:PAD], in_=zero_sb[:, 0:PAD])
        nc.gpsimd.dma_start(out=pad_d[:, TP - PAD : TP], in_=zero_sb[:, 0:PAD])
        nc.gpsimd.dma_start(out=pad_d[0:BL, PAD : PAD + T], in_=aw_d[:])
        nc.gpsimd.dma_start(out=pad_d[BL : 2 * BL, PAD : PAD + T], in_=awcb_d[:])
        im2 = cpool.tile([2 * K, BL * T], BF16, tag="im2")
        for c in range(2):
            src = _dc.replace(
                pad_d[:], ap=[[1, K], [TP, BL], [1, T]], offset=c * BL * TP
            )
            nc.gpsimd.dma_start(
                out=im2[c * K : (c + 1) * K, :].rearrange("k (b t) -> k b t", b=BL),
                in_=src,
            )
        memTs_p2 = {0: load_pair(smem_d, 0, nc.gpsimd)}

        # sel[:, b*128:(b+1)*128] is an [8, 128] selector whose row b is all
        # ones: bc = sel_b.T @ w_bf broadcasts w row b across 128 partitions.
        # Built with two affine selects on the gpsimd queue, slotted after
        # smemT1 so neither the conv path nor the bc matmuls wait on it:
        # keep 1.0 where 0 <= j - 128*p <= 127, else fill 0.
        sel = cpool.tile([BL, BL * 128], BF16, tag="sel")
        nc.gpsimd.memset(sel[:], 1.0)
        nc.gpsimd.affine_select(
            out=sel[:],
            in_=sel[:],
            compare_op=ALU.is_ge,
            fill=0.0,
            base=0,
            pattern=[[1, BL * 128]],
            channel_multiplier=-128,
        )
        nc.gpsimd.affine_select(
            out=sel[:],
            in_=sel[:],
            compare_op=ALU.is_ge,
            fill=0.0,
            base=127,
            pattern=[[-1, BL * 128]],
            channel_multiplier=128,
        )

        for pr in range(1, BL // 2):
            memTs_p2[pr] = load_pair(smem_d, pr, nc.gpsimd)

        # ---------------- constants / on-chip weight prep ----------------
        # setup copies run on DVE (idle until the phase-1 chain) so the ACT
        # queue reaches conv_s/tanh — and thus sigmoid1 — as early as possible
        def transpose_into(dst, nat, ncols, dt, eng_copy, interleave=False):
            idn = {F32: ident, BF16: ident_bf}[dt]
            for c in range(ncols // 128):
                if interleave:
                    # column set {4j + c}: gives W.T rows for the "(p c) t"
                    # memory layout where E row 4p+c sits at partition p
                    sl = nat[:, c : c + 1]
                    src = _dc.replace(sl, ap=[sl.ap[0], [4, 128]])
                else:
                    src = nat[:, c * 128 : (c + 1) * 128]
                tp = ppm.tile([128, 512], dt, tag="pm", name="wtp")
                nc.tensor.transpose(tp[:, 0:128], src, idn[:])
                if eng_copy is nc.scalar:
                    nc.scalar.activation(
                        dst[:, c * 128 : (c + 1) * 128], tp[:, 0:128], AF.Copy
                    )
                else:
                    eng_copy.tensor_scalar_add(
                        dst[:, c * 128 : (c + 1) * 128], tp[:, 0:128], 0.0
                    )

        mWT = cpool.tile([128, E], F8, tag="mWT")
        transpose_into(mWT, mW_nat, E, BF16, nc.scalar, interleave=True)
        qWT = cpool.tile([128, RNN], BF16, tag="qWT")
        transpose_into(qWT, qW_nat, RNN, BF16, nc.vector)
        # smWT/sqWT/spq are deferred into the phase loop (issued after the
        # phase-1 rows) so their transposes don't delay the first pm

        qT = cpool.tile([128, 8 * BL], BF16, tag="qT")  # cols (rchunk, b)
        for c in range(8):
            tp = ppm.tile([128, 512], F32, tag="pm", name="qtp")
            nc.tensor.transpose(
                tp[:, 0:BL], q_sb[:, c * 128 : (c + 1) * 128], ident[0:BL, 0:BL]
            )
            nc.vector.tensor_scalar_add(qT[:, c * BL : (c + 1) * BL], tp[:, 0:BL], 0.0)

        convWT = cpool.tile([2 * K, NF], BF16, tag="convWT")
        nc.vector.tensor_scalar_add(convWT[:], convWT_f[:], 0.0)
        ldWT = cpool.tile([NF, AD], BF16, tag="ldWT")  # x64 to match fp8 weight scale
        nc.vector.tensor_scalar_mul(ldWT[:], ldWT_f[:], WS)

        # vmat[:, bi*BL + j] = v if j == bi else 0 — row-masked v so the
        # e contraction for row bi lands in PSUM partition bi.
        def masked_v(col, name):
            t = cpool.tile([128, BL * BL], BF16, tag=name)
            nc.vector.memset(t[:], 0.0)
            for bi in range(BL):
                c = bi * BL + bi
                nc.vector.tensor_scalar_add(
                    t[:, c : c + 1], v_sb[:, col : col + 1], 0.0
                )
            return t

        vmat = masked_v(0, "vmat")
        svmat = masked_v(1, "svmat")

        # context / u_new staging for the whole local batch (partitions 0..BL)
        out_tile = cpool.tile([BL, OUT_W], F32, tag="out_tile")
        scr8 = out_tile[0:BL, CUM0 : CUM0 + RNN]  # never reaches DRAM from here

        # ---------------- query projections (pq, spq) ----------------
        def project_query(wT, name):
            ps = ppm.tile([128, 512], F32, tag="pm", name="pq_ps")
            for rc in range(8):
                nc.tensor.matmul(
                    ps[:, 0:BL],
                    lhsT=wT[:, rc * 128 : (rc + 1) * 128],
                    rhs=qT[:, rc * BL : (rc + 1) * BL],
                    start=(rc == 0),
                    stop=(rc == 7),
                )
            sb = cpool.tile([128, BL], F32, tag=name)
            nc.vector.tensor_scalar_add(sb[:], ps[:, 0:BL], 0.0)
            return sb

        pq_sb = project_query(qWT, "pq_sb")
        smWT = sqWT = spq_sb = None  # created after the first ph1 row issues

        # ---------------- per-phase pieces ----------------
        def row_matmuls(memTs, wT, pq, v, with_loc, e_segs, b):
            """conv (optional) + pm (fp8 DoubleRow) + tanh + e-matmul for row b."""
            memT, roff = pair_view(memTs, b)
            wT3 = wT[:].rearrange("p (g m) -> p g m", g=4)  # 4 k-tiles of 128
            if with_loc:
                cps = ppm.tile([128, 512], F32, tag="pm", name="cps")
                conv_s = cspool.tile([NF, T], BF16, tag="convs", name="conv_s")
                for si, (t0, tl) in enumerate(SEGS):
                    nc.tensor.matmul(
                        cps[0:NF, 0:tl],
                        lhsT=convWT[:],
                        rhs=im2[:, b * T + t0 : b * T + t0 + tl],
                        start=True,
                        stop=True,
                        skip_group_check=True,
                    )
                    # split the psum->sbuf bounce across ACT and DVE so
                    # neither paces the row stream
                    if si == 0:
                        nc.scalar.activation(
                            conv_s[:, t0 : t0 + tl], cps[0:NF, 0:tl], AF.Copy
                        )
                    else:
                        nc.vector.tensor_scalar_add(
                            conv_s[:, t0 : t0 + tl], cps[0:NF, 0:tl], 0.0
                        )
            th = tpool.tile([128, T], BF16, tag="tanh", name="th")
            for si, (t0, tl) in enumerate(SEGS):
                pm = ppm.tile([128, 512], F32, tag="pm", name="pm")
                for g in range(2):  # two DoubleRow matmuls cover 4 k-tiles
                    sl = memT[:, roff + 2 * g * T + t0 : roff + 2 * g * T + t0 + tl]
                    rhs = _dc.replace(sl, ap=[sl.ap[0], [T, 2], [1, tl]])
                    nc.tensor.matmul(
                        pm[:, 0:tl],
                        lhsT=wT3[:, 2 * g : 2 * g + 2, :],
                        rhs=rhs,
                        start=(g == 0),
                        stop=(g == 1 and not with_loc),
                        perf_mode=PM_DR,
                        skip_group_check=True,
                    )
                if with_loc:
                    nc.tensor.matmul(
                        pm[:, 0:tl],
                        lhsT=ldWT[:],
                        rhs=conv_s[:, t0 : t0 + tl],
                        start=False,
                        stop=True,
                        skip_group_check=True,
                    )
                nc.scalar.activation(
                    th[:, t0 : t0 + tl],
                    pm[:, 0:tl],
                    AF.Tanh,
                    bias=pq[:, b : b + 1],
                    scale=1.0 / WS,
                )
                nc.tensor.matmul(
                    e_segs[si][0:BL, 0:tl],
                    lhsT=v[:, b * BL : (b + 1) * BL],
                    rhs=th[:, t0 : t0 + tl],
                    start=(b == 0),
                    stop=(b == BL - 1),
                    skip_group_check=True,
                )

        def make_esegs(name):
            return [
                ppe.tile([BL, 512], F32, tag=f"pe{si}", name=f"{name}{si}")
                for si in range(2)
            ]

        def phase_chain(e_segs, with_loc, w_rt, w_bf, sig_rt):
            """sigmoid + normalization (+ alpha recurrence for phase 1)."""
            for si, (t0, tl) in enumerate(SEGS):
                nc.scalar.activation(
                    sig_rt[:, t0 : t0 + tl],
                    e_segs[si][0:BL, 0:tl],
                    AF.Sigmoid,
                    accum_out=cols[:, 6 + si : 7 + si],
                )
            if with_loc:
                nc.vector.tensor_add(cols[:, 8:9], cols[:, 6:7], cols[:, 7:8])
                nc.vector.reciprocal(cols[:, 9:10], cols[:, 8:9])
                # cum_new = awc + sig/sum(sig) in one fused op
                anew_rt = rtp.tile([BL, T], F32, tag="anew_rt", name="anew_rt")
                nc.vector.scalar_tensor_tensor(
                    out=anew_rt[:],
                    in0=sig_rt[:],
                    scalar=cols[:, 9:10],
                    in1=awc_rt[:],
                    op0=ALU.mult,
                    op1=ALU.add,
                )
                nc.sync.dma_start(out=out_d[:, CUM0 : CUM0 + T], in_=anew_rt[:])
                # monotonic alpha recurrence; the sigmoid-normalizing scalar
                # cancels in alignments = x/sum(x), so run the chain off raw
                # sig:  base = (1-u)*alpha + u*shift(alpha)
                #       w    = (base + 1e-8)*sig, normalized
                nc.vector.tensor_scalar(
                    out=cols[:, 1:2],
                    in0=cols[:, 0:1],
                    scalar1=-1.0,
                    scalar2=1.0,
                    op0=ALU.mult,
                    op1=ALU.add,
                )
                shift_rt = rtp.tile([BL, T], F32, tag="shift_rt", name="shift_rt")
                base_rt = rtp.tile([BL, T], F32, tag="base_rt", name="base_rt")
                nc.vector.memset(shift_rt[:, 0:1], 0.0)
                nc.vector.tensor_scalar_mul(
                    shift_rt[:, 1:T], alpha_rt[:, 0 : T - 1], cols[:, 0:1]
                )
                nc.vector.scalar_tensor_tensor(
                    out=base_rt[:],
                    in0=alpha_rt[:],
                    scalar=cols[:, 1:2],
                    in1=shift_rt[:],
                    op0=ALU.mult,
                    op1=ALU.add,
                )
                nc.vector.scalar_tensor_tensor(
                    out=base_rt[:],
                    in0=base_rt[:],
                    scalar=1e-8,
                    in1=sig_rt[:],
                    op0=ALU.add,
                    op1=ALU.mult,
                    accum_out=cols[:, 2:3],
                )
                nc.vector.reciprocal(cols[:, 3:4], cols[:, 2:3])
                nc.vector.tensor_scalar_mul(w_rt[:], base_rt[:], cols[:, 3:4])
            else:
                nc.vector.tensor_add(cols[:, 8:9], cols[:, 6:7], cols[:, 7:8])
                nc.vector.reciprocal(cols[:, 9:10], cols[:, 8:9])
                nc.vector.tensor_scalar_mul(w_rt[:], sig_rt[:], cols[:, 9:10])
            nc.scalar.activation(w_bf[:], w_rt[:], AF.Copy)

        def row_ctx(memTs, w_bf, ctxT, b):
            """bc broadcast matmul + 4 full-T weighted-sum accumulations.

            DVE rows: scalar_tensor_tensor with f32 accum (verified 2e-6).
            Pool rows: gpsimd tensor_tensor f8 x bf16 -> f32 product, then
            ACT Copy with accum_out (f32 accumulation follows the f32 input;
            gpsimd STT is rejected by codegen and ACT accumulation over a
            bf16 input is only ~1e-1 accurate).
            bc is bounced to bf16 SBUF for everyone: gpsimd cannot read
            PSUM, and the f8 x f32 STT path accumulates poorly.
            """
            memT, roff = pair_view(memTs, b)
            bc = pbc.tile([128, 1024], F32, tag="bc", name="bc")
            for t0, tl in SEGS:
                nc.tensor.matmul(
                    bc[:, t0 : t0 + tl],
                    lhsT=sel[:, b * 128 : (b + 1) * 128],
                    rhs=w_bf[:, t0 : t0 + tl],
                    start=True,
                    stop=True,
                    skip_group_check=True,
                )
            bc_sb = bsbpool.tile([128, T], BF16, tag="bcsb", name="bc_sb")
            nc.scalar.activation(bc_sb[:], bc[:, 0:T], AF.Copy)
            for c in range(4):
                cc = c * BL + b
                if b in DVE_ROWS:
                    scr = scpool_v.tile([128, T], BF16, tag="scr", name="scr")
                    nc.vector.scalar_tensor_tensor(
                        out=scr[:],
                        in0=memT[:, roff + c * T : roff + (c + 1) * T],
                        scalar=1.0,
                        in1=bc_sb[:],
                        op0=ALU.mult,
                        op1=ALU.mult,
                        accum_out=ctxT[:, cc : cc + 1],
                    )
                else:
                    scr32 = scpool_p.tile([128, T], F32, tag="scr32", name="scr32")
                    nc.gpsimd.tensor_tensor(
                        scr32[:],
                        memT[:, roff + c * T : roff + (c + 1) * T],
                        bc_sb[:],
                        ALU.mult,
                    )
                    dump = scpool_a.tile([128, T], BF16, tag="dump", name="dump")
                    nc.scalar.activation(
                        dump[:], scr32[:], AF.Copy, accum_out=ctxT[:, cc : cc + 1]
                    )

        def ctx_finalize(ctxT, ctx_off):
            # ctxT col (c, b) holds ctx[b, e] for e = 4p + c (interleaved
            # memory layout) -> transpose chunk c lands in out cols c::4
            for c in range(4):
                tp = ppm.tile([128, 512], F32, tag="pm", name="ctp")
                nc.tensor.transpose(
                    tp[0:BL, 0:128], ctxT[:, c * BL : (c + 1) * BL], ident[:]
                )
                sl = out_tile[0:BL, ctx_off + c : ctx_off + c + 1]
                dst = _dc.replace(sl, ap=[sl.ap[0], [4, 128]])
                nc.scalar.activation(dst, tp[0:BL, 0:128], AF.Copy)

        for _rep in range(repeat):
            if _rep > 0:
                memTs_p1 = {p: load_pair(mem_d, p, nc.sync) for p in range(BL // 2)}
                memTs_p2 = {p: load_pair(smem_d, p, nc.gpsimd) for p in range(BL // 2)}

            # ---- phase 1: location-sensitive monotonic attention ----
            e1 = make_esegs("e1")
            for b in range(BL):
                row_matmuls(memTs_p1, mWT, pq_sb, vmat, True, e1, b)
                if b == 1 and smWT is None:
                    # slot the phase-2 weight prep into the row stream's PE
                    # slack (rows are ACT/DVE-paced)
                    smWT = cpool.tile([128, EL], F8, tag="smWT")
                    transpose_into(smWT, smW_nat, EL, BF16, nc.scalar, interleave=True)
                    sqWT = cpool.tile([128, RNN], BF16, tag="sqWT")
                    transpose_into(sqWT, sqW_nat, RNN, BF16, nc.vector)
                    spq_sb = project_query(sqWT, "spq_sb")
            sig1 = rtp.tile([BL, T], F32, tag="sig1", name="sig1")
            w1 = rtp.tile([BL, T], F32, tag="w1", name="w1")
            w1_bf = rtp.tile([BL, T], BF16, tag="w1bf", name="w1_bf")
            phase_chain(e1, True, w1, w1_bf, sig1)
            nc.sync.dma_start(out=out_d[:, ALIGN0 : ALIGN0 + T], in_=w1[:])

            ctxT1 = cpool.tile([128, 4 * BL], F32, tag="ctxT1")
            e2 = make_esegs("e2")
            # interleave phase-2 row matmuls with phase-1 ctx so the PE queue
            # never parks behind a bc matmul waiting on DVE/Pool
            for b in range(BL):
                row_matmuls(memTs_p2, smWT, spq_sb, svmat, False, e2, b)
                row_ctx(memTs_p1, w1_bf, ctxT1, b)
            ctx_finalize(ctxT1, CTX0)

            # u_new = sigmoid([context, query] @ ta_W.T + ta_b)
            nc.vector.scalar_tensor_tensor(
                out=scr8[:, 0:E],
                in0=out_tile[0:BL, CTX0 : CTX0 + E],
                scalar=1.0,
                in1=taWb[:, 0:E],
                op0=ALU.mult,
                op1=ALU.mult,
                accum_out=cols[:, 4:5],
            )
            nc.vector.scalar_tensor_tensor(
                out=scr8[:, 0:RNN],
                in0=q_sb[:],
                scalar=1.0,
                in1=taWb[:, E : E + RNN],
                op0=ALU.mult,
                op1=ALU.mult,
                accum_out=cols[:, 10:11],
            )
            nc.vector.tensor_add(cols[:, 11:12], cols[:, 4:5], cols[:, 10:11])
            nc.scalar.activation(
                out_tile[0:BL, UN0 : UN0 + 1],
                cols[:, 11:12],
                AF.Sigmoid,
                bias=cols[:, 5:6],
            )

            # ---- phase 2: additive self-attention ----
            sig2 = rtp.tile([BL, T], F32, tag="sig2", name="sig2")
            w2 = rtp.tile([BL, T], F32, tag="w2", name="w2")
            w2_bf = rtp.tile([BL, T], BF16, tag="w2bf", name="w2_bf")
            phase_chain(e2, False, w2, w2_bf, sig2)
            nc.sync.dma_start(out=out_d[:, W2_0 : W2_0 + T], in_=w2[:])

            ctxT2 = cpool.tile([128, 4 * BL], F32, tag="ctxT2")
            for b in range(BL):
                row_ctx(memTs_p2, w2_bf, ctxT2, b)
            ctx_finalize(ctxT2, CTX2_0)

            nc.sync.dma_start(
                out=out_d[:, CTX0 : CTX0 + E], in_=out_tile[:, CTX0 : CTX0 + E]
            )
            nc.sync.dma_start(
                out=out_d[:, UN0 : UN0 + 1], in_=out_tile[:, UN0 : UN0 + 1]
            )
            nc.sync.dma_start(
                out=out_d[:, CTX2_0 : CTX2_0 + EL],
                in_=out_tile[:, CTX2_0 : CTX2_0 + EL],
            )

    if finalize:
        nc.finalize()
    return nc


_NC = None
RUN_KWARGS: dict = {}   # test harness can set {"trace": True}
LAST_RESULT = None      # BassKernelResults of the most recent kernel() call


def _get_nc():
    global _NC
    if _NC is None:
        _NC = build_nc()
    return _NC


def make_in_map(shard: dict) -> dict:
    """Device in_map for ONE core's shard (keys as in setup_inputs)."""
    f = lambda k: np.ascontiguousarray(np.asarray(shard[k], dtype=np.float32))
    bf = ml_dtypes.bfloat16
    f8 = ml_dtypes.float8_e4m3
    return {
        "query": f("query"),
        "memory": np.ascontiguousarray(f("memory").transpose(0, 2, 1).astype(f8)),
        "self_memory": np.ascontiguousarray(
            f("self_memory").transpose(0, 2, 1).astype(f8)
        ),
        "attention_weights": f("attention_weights").astype(bf),
        "attention_weights_cum": f("attention_weights_cum"),
        "awc_bf": f("attention_weights_cum").astype(bf),
        "alpha": f("alpha"),
        "u": f("u"),
        "memory_W": (f("memory_W") * WS).astype(bf),
        "query_W": f("query_W").astype(bf),
        "v_W": f("v_W"),
        "loc_conv_W": f("loc_conv_W"),
        "loc_dense_W": f("loc_dense_W"),
        "ta_W": f("ta_W"),
        "ta_b": f("ta_b").reshape(1, 1),
        "self_memory_W": (f("self_memory_W") * WS).astype(bf),
        "self_query_W": f("self_query_W").astype(bf),
        "self_v_W": f("self_v_W"),
    }


def kernel(**inputs) -> np.ndarray:
    f = lambda k: np.ascontiguousarray(np.asarray(inputs[k], dtype=np.float32))
    bf = ml_dtypes.bfloat16
    f8 = ml_dtypes.float8_e4m3
    rep = {
        "memory_W": (f("memory_W") * WS).astype(bf),
        "query_W": f("query_W").astype(bf),
        "v_W": f("v_W"),
        "loc_conv_W": f("loc_conv_W"),
        "loc_dense_W": f("loc_dense_W"),
        "ta_W": f("ta_W"),
        "ta_b": f("ta_b").reshape(1, 1),
        "self_memory_W": (f("self_memory_W") * WS).astype(bf),
        "self_query_W": f("self_query_W").astype(bf),
        "self_v_W": f("self_v_W"),
    }
    mem_t = np.ascontiguousarray(f("memory").transpose(0, 2, 1).astype(f8))
    smem_t = np.ascontiguousarray(f("self_memory").transpose(0, 2, 1).astype(f8))
    aw_bf = f("attention_weights").astype(bf)
    awc = f("attention_weights_cum")
    awc_bf = awc.astype(bf)
    q = f("query")
    alpha = f("alpha")
    u = f("u")
    in_maps = []
    for i in range(NCORES):
        sl = slice(i * BL, (i + 1) * BL)
        m = dict(rep)
        m["query"] = q[sl]
        m["memory"] = mem_t[sl]
        m["self_memory"] = smem_t[sl]
        m["attention_weights"] = aw_bf[sl]
        m["attention_weights_cum"] = awc[sl]
        m["awc_bf"] = awc_bf[sl]
        m["alpha"] = alpha[sl]
        m["u"] = u[sl]
        in_maps.append(m)
    global LAST_RESULT
    res = run_bass_kernel_spmd(
        _get_nc(), in_maps, core_ids=list(range(NCORES)), **RUN_KWARGS
    )
    LAST_RESULT = res
    return np.concatenate([res.results[i]["out"] for i in range(NCORES)], axis=0)


# revision 40
# speedup vs baseline: 3.6971x; 1.3401x over previous
"""Trainium2 Bass kernel for nn_Attention_15410342658774 (v2).

Location-sensitive monotonic attention + additive self-attention
(Tacotron-style), B=64, T=1000, E=EL=512, RNN=1024, AD=128.

Pure data parallel across 8 NeuronCores (8 batch rows each, weights
replicated).  Host pre-transposes `memory`/`self_memory` to [B, E, T]
and casts to fp8e4 (halves the HBM stream vs bf16 and enables DoubleRow
matmuls); weight matrices ride along as fp8e4 scaled by 64 (folded back
out via the tanh activation's input scale).

v2 changes vs the 162us baseline (cost-model-driven):
  - fp8e4 memory stream: DMA_ENGINES floor 56us -> ~28us; DoubleRow
    fp8 matmuls for pm (2 k-tiles per instruction at 0.5 cyc/row).
  - All big DMAs on dedicated queues (sync for phase 1 + weights,
    gpsimd for phase 2 + im2col) - never on the ACT/DVE/PE queues,
    since a dma_start occupies its queue for the whole transfer.
  - ctx accumulation (the 82us DVE hotspot; scalar_tensor_tensor has
    no DVE fast mode) split DVE/Pool: DVE rows read the bc broadcast
    straight from PSUM, Pool rows get an ACT-copied bf16 SBUF view
    (gpsimd cannot touch PSUM).
  - bc broadcast matmuls in bf16 (they were f32 = 4 cyc/row on PE).
  - reductions fused into producers via accum_out (sigmoid-sum,
    alpha-chain sum).
  - PSUM budget reworked to exactly 8 banks: pm 2 + bc 2x2 + e-segs 2.

Hardware constraints baked in (from the v1 session + cost model):
  - matmul operands/outputs must start at partition 0/32/64; PSUM
    matmul outputs must not cross a 2KB bank.
  - DMA access patterns: at most 3 [step,count] dims, innermost step 1.
  - TENSOR_TENSOR_REDUCE crashes the exec unit; scalar_tensor_tensor's
    accum_out is the working per-partition reduction.
  - gpsimd (Pool) engine: SBUF only, no PSUM access.
"""

import dataclasses as _dc
import sys

import numpy as np

_TRN = "/opt/trn_rl_repo"
if _TRN not in sys.path:
    sys.path.insert(0, _TRN)

from contextlib import ExitStack

import ml_dtypes

import concourse.bacc as bacc
import concourse.bass as bass
import concourse.mybir as mybir
from concourse.bass_utils import run_bass_kernel_spmd
from concourse.masks import make_identity
from concourse.tile import TileContext

B, T = 64, 1000
E, EL, RNN, AD = 512, 512, 1024, 128
NF, K = 32, 31
PAD = (K - 1) // 2
NCORES = 8
BL = B // NCORES  # 8 batch rows per core
F32 = mybir.dt.float32
BF16 = mybir.dt.bfloat16
F8 = mybir.dt.float8e4
AF = mybir.ActivationFunctionType
ALU = mybir.AluOpType
AX = mybir.AxisListType
PM_DR = mybir.MatmulPerfMode.DoubleRow
SEGS = [(0, 512), (512, 488)]  # T split at the 512-float PSUM bank boundary
WS = 64.0  # fp8 weight pre-scale (exact power of two)

# output packing: [context(E) | alignments(T) | u_new(1) | cum_new(T) | ctx2(EL) | w2(T)]
CTX0 = 0
ALIGN0 = E
UN0 = E + T
CUM0 = E + T + 1
CTX2_0 = E + 2 * T + 1
W2_0 = E + EL + 2 * T + 1
OUT_W = E + EL + 3 * T + 1  # 4025

DVE_ROWS = (0, 1, 2, 3, 4, 5, 6, 7)  # all ctx rows on DVE (Pool TT was slow on HW)


def build_nc(finalize: bool = True, repeat: int = 1) -> bass.Bass:
    nc = bacc.Bacc()

    q_d = nc.declare_dram_parameter("query", [BL, RNN], F32, isOutput=False)
    # pre-transposed [E, T] per row, fp8e4 (host-prepared)
    mem_d = nc.declare_dram_parameter("memory", [BL, E, T], F8, isOutput=False)
    smem_d = nc.declare_dram_parameter("self_memory", [BL, EL, T], F8, isOutput=False)
    aw_d = nc.declare_dram_parameter("attention_weights", [BL, T], BF16, isOutput=False)
    awc_d = nc.declare_dram_parameter(
        "attention_weights_cum", [BL, T], F32, isOutput=False
    )
    awcb_d = nc.declare_dram_parameter("awc_bf", [BL, T], BF16, isOutput=False)
    al_d = nc.declare_dram_parameter("alpha", [BL, T], F32, isOutput=False)
    u_d = nc.declare_dram_parameter("u", [BL, 1], F32, isOutput=False)
    mW_d = nc.declare_dram_parameter("memory_W", [AD, E], BF16, isOutput=False)  # x64
    qW_d = nc.declare_dram_parameter("query_W", [AD, RNN], BF16, isOutput=False)
    vW_d = nc.declare_dram_parameter("v_W", [1, AD], F32, isOutput=False)
    cW_d = nc.declare_dram_parameter("loc_conv_W", [NF, 2, K], F32, isOutput=False)
    dW_d = nc.declare_dram_parameter("loc_dense_W", [AD, NF], F32, isOutput=False)
    taW_d = nc.declare_dram_parameter("ta_W", [1, E + RNN], F32, isOutput=False)
    tab_d = nc.declare_dram_parameter("ta_b", [1, 1], F32, isOutput=False)
    smW_d = nc.declare_dram_parameter("self_memory_W", [AD, EL], BF16, isOutput=False)
    sqW_d = nc.declare_dram_parameter("self_query_W", [AD, RNN], BF16, isOutput=False)
    svW_d = nc.declare_dram_parameter("self_v_W", [1, AD], F32, isOutput=False)
    out_d = nc.declare_dram_parameter("out", [BL, OUT_W], F32, isOutput=True)

    with ExitStack() as ctx:
        tc = ctx.enter_context(TileContext(nc))
        cpool = ctx.enter_context(tc.tile_pool(name="const", bufs=1))
        mpool = ctx.enter_context(tc.tile_pool(name="mem", bufs=8))
        tpool = ctx.enter_context(tc.tile_pool(name="tanhp", bufs=3))
        cspool = ctx.enter_context(tc.tile_pool(name="convsp", bufs=2))
        bsbpool = ctx.enter_context(tc.tile_pool(name="bsb", bufs=3))
        # separate STT scratch pools per engine — a shared pool would make
        # every row's scratch reuse the other engine's slots, serializing the
        # DVE and Pool ctx streams into lockstep
        scpool_v = ctx.enter_context(tc.tile_pool(name="scrv", bufs=2))
        scpool_p = ctx.enter_context(tc.tile_pool(name="scrp", bufs=2))
        scpool_a = ctx.enter_context(tc.tile_pool(name="scra", bufs=2))
        rtp = ctx.enter_context(tc.tile_pool(name="rtp", bufs=1))
        ldp = ctx.enter_context(tc.tile_pool(name="ldp", bufs=2))
        # PSUM: 8 banks of [128, 512] f32. pm/conv 4 (2 banks spare) + e-segs 2.
        ppm = ctx.enter_context(tc.tile_pool(name="ppm", bufs=4, space="PSUM"))
        ppe = ctx.enter_context(tc.tile_pool(name="ppe", bufs=1, space="PSUM"))

        # identity goes FIRST on the gpsimd queue — everything transposes
        # through it, and gpsimd also carries the phase-2 DMA stream
        ident = cpool.tile([128, 128], F32, tag="ident")
        make_identity(nc, ident[:])
        ident_bf = cpool.tile([128, 128], BF16, tag="ident_bf")
        nc.scalar.activation(ident_bf[:], ident[:], AF.Copy)

        # ---------------- DMA issue: weights + phase-1 memory on sync,
        # ---------------- im2col + phase-2 memory on gpsimd ----------------
        # Memory rides in the "(p c) t" interleaved layout: partition p holds
        # E rows 4p..4p+3 as one contiguous 4000B run per row, so one DMA can
        # carry TWO batch rows in a 3-dim access pattern (the per-DMA fixed
        # cost - dge + semaphore - was pacing each queue to one 0.5MB tile
        # per ~3us).  E row 4p+c lives at col (b%2)*4T + c*T.
        def load_pair(mem_dram, pair, eng):
            memT = mpool.tile([128, 8 * T], F8, tag="memT", name="memT")
            src = _dc.replace(
                mem_dram[:],
                ap=[[4 * T, 128], [E * T, 2], [1, 4 * T]],
                offset=pair * 2 * E * T,
            )
            eng.dma_start(
                out=memT[:].rearrange("p (b x) -> p b x", b=2),
                in_=src,
            )
            return memT

        def pair_view(memTs, b):
            return memTs[b // 2], (b % 2) * 4 * T

        mW_nat = ldp.tile([AD, E], BF16, tag="mw_nat", name="mW_nat")
        nc.sync.dma_start(out=mW_nat[:], in_=mW_d[:])
        memTs_p1 = {0: load_pair(mem_d, 0, nc.sync)}
        qW_nat = ldp.tile([AD, RNN], BF16, tag="qw_nat", name="qW_nat")
        nc.sync.dma_start(out=qW_nat[:], in_=qW_d[:])
        q_sb = cpool.tile([BL, RNN], F32, tag="q_sb")
        nc.sync.dma_start(out=q_sb[:], in_=q_d[:])
        memTs_p1[1] = load_pair(mem_d, 1, nc.sync)
        convWT_f = cpool.tile([2 * K, NF], F32, tag="convWT_f")  # [(c k), o]
        nc.sync.dma_start(out=convWT_f[:], in_=cW_d[:].rearrange("o c k -> (c k) o"))
        ldWT_f = cpool.tile([NF, AD], F32, tag="ldWT_f")
        nc.sync.dma_start(out=ldWT_f[:], in_=dW_d[:].rearrange("a f -> f a"))
        v_sb = cpool.tile([128, 2], F32, tag="v_sb")
        nc.sync.dma_start(out=v_sb[:, 0:1], in_=vW_d[:].rearrange("o a -> a o"))
        nc.sync.dma_start(out=v_sb[:, 1:2], in_=svW_d[:].rearrange("o a -> a o"))
        memTs_p1[2] = load_pair(mem_d, 2, nc.sync)

        # per-row scalars: 0=u 1=1-u 2..3 alpha-sums 4=s_ta1 5=ta_b 6..7 sig-sums
        cols = cpool.tile([BL, 12], F32, tag="cols")
        nc.sync.dma_start(
            out=cols[:, 5:6], in_=_dc.replace(tab_d[:], ap=[[0, BL], [1, 1]])
        )
        awc_rt = rtp.tile([BL, T], F32, tag="awc_rt", name="awc_rt")
        nc.sync.dma_start(out=awc_rt[:], in_=awc_d[:])
        memTs_p1[3] = load_pair(mem_d, 3, nc.sync)

        smW_nat = ldp.tile([AD, EL], BF16, tag="smw_nat", name="smW_nat")
        nc.sync.dma_start(out=smW_nat[:], in_=smW_d[:])
        sqW_nat = ldp.tile([AD, RNN], BF16, tag="sqw_nat", name="sqW_nat")
        nc.sync.dma_start(out=sqW_nat[:], in_=sqW_d[:])
        taWb = cpool.tile([BL, E + RNN], F32, tag="taWb")
        nc.sync.dma_start(
            out=taWb[:], in_=_dc.replace(taW_d[:], ap=[[0, BL], [1, E + RNN]])
        )
        alpha_rt = rtp.tile([BL, T], F32, tag="alpha_rt", name="alpha_rt")
        nc.sync.dma_start(out=alpha_rt[:], in_=al_d[:])
        nc.sync.dma_start(out=cols[:, 0:1], in_=u_d[:])

        # im2col of [attention_weights; attention_weights_cum] for the
        # location conv: rows (c, k), cols (b, t); zero-padded edges via a
        # DRAM bounce so the sliding window is 2 DMAs.  All on gpsimd.
        TP = T + 2 * PAD
        pad_d = nc.dram_tensor("awc_pad", [2 * BL, TP], BF16)
        zero_sb = cpool.tile([2 * BL, 16], BF16, tag="zero_sb")
        nc.vector.memset(zero_sb[:], 0.0)
        nc.gpsimd.dma_start(out=pad_d[:, 0:PAD], in_=zero_sb[:, 0:PAD])
        nc.gpsimd.dma_start(out=pad_d[:, TP - PAD : TP], in_=zero_sb[:, 0:PAD])
        nc.gpsimd.dma_start(out=pad_d[0:BL, PAD : PAD + T], in_=aw_d[:])
        nc.gpsimd.dma_start(out=pad_d[BL : 2 * BL, PAD : PAD + T], in_=awcb_d[:])
        im2 = cpool.tile([2 * K, BL * T], BF16, tag="im2")
        for c in range(2):
            src = _dc.replace(
                pad_d[:], ap=[[1, K], [TP, BL], [1, T]], offset=c * BL * TP
            )
            nc.gpsimd.dma_start(
                out=im2[c * K : (c + 1) * K, :].rearrange("k (b t) -> k b t", b=BL),
                in_=src,
            )
        memTs_p2 = {0: load_pair(smem_d, 0, nc.gpsimd)}

        for pr in range(1, BL // 2):
            memTs_p2[pr] = load_pair(smem_d, pr, nc.gpsimd)

        # ---------------- constants / on-chip weight prep ----------------
        # setup copies run on DVE (idle until the phase-1 chain) so the ACT
        # queue reaches conv_s/tanh — and thus sigmoid1 — as early as possible
        def transpose_into(dst, nat, ncols, dt, eng_copy, interleave=False):
            idn = {F32: ident, BF16: ident_bf}[dt]
            for c in range(ncols // 128):
                if interleave:
                    # column set {4j + c}: gives W.T rows for the "(p c) t"
                    # memory layout where E row 4p+c sits at partition p
                    sl = nat[:, c : c + 1]
                    src = _dc.replace(sl, ap=[sl.ap[0], [4, 128]])
                else:
                    src = nat[:, c * 128 : (c + 1) * 128]
                tp = ppm.tile([128, 512], dt, tag="pm", name="wtp")
                nc.tensor.transpose(tp[:, 0:128], src, idn[:])
                if eng_copy is nc.scalar:
                    nc.scalar.activation(
                        dst[:, c * 128 : (c + 1) * 128], tp[:, 0:128], AF.Copy
                    )
                else:
                    eng_copy.tensor_scalar_add(
                        dst[:, c * 128 : (c + 1) * 128], tp[:, 0:128], 0.0
                    )

        mWT = cpool.tile([128, E], F8, tag="mWT")
        transpose_into(mWT, mW_nat, E, BF16, nc.scalar, interleave=True)
        qWT = cpool.tile([128, RNN], BF16, tag="qWT")
        transpose_into(qWT, qW_nat, RNN, BF16, nc.vector)
        # smWT/sqWT/spq are deferred into the phase loop (issued after the
        # phase-1 rows) so their transposes don't delay the first pm

        qT = cpool.tile([128, 8 * BL], BF16, tag="qT")  # cols (rchunk, b)
        for c in range(8):
            tp = ppm.tile([128, 512], F32, tag="pm", name="qtp")
            nc.tensor.transpose(
                tp[:, 0:BL], q_sb[:, c * 128 : (c + 1) * 128], ident[0:BL, 0:BL]
            )
            nc.vector.tensor_scalar_add(qT[:, c * BL : (c + 1) * BL], tp[:, 0:BL], 0.0)

        convWT = cpool.tile([2 * K, NF], BF16, tag="convWT")
        nc.vector.tensor_scalar_add(convWT[:], convWT_f[:], 0.0)
        ldWT = cpool.tile([NF, AD], BF16, tag="ldWT")  # x64 to match fp8 weight scale
        nc.vector.tensor_scalar_mul(ldWT[:], ldWT_f[:], WS)

        # vmat[:, bi*BL + j] = v if j == bi else 0 — row-masked v so the
        # e contraction for row bi lands in PSUM partition bi.
        def masked_v(col, name):
            t = cpool.tile([128, BL * BL], BF16, tag=name)
            nc.vector.memset(t[:], 0.0)
            for bi in range(BL):
                c = bi * BL + bi
                nc.vector.tensor_scalar_add(
                    t[:, c : c + 1], v_sb[:, col : col + 1], 0.0
                )
            return t

        vmat = masked_v(0, "vmat")
        svmat = masked_v(1, "svmat")

        # context / u_new staging for the whole local batch (partitions 0..BL)
        out_tile = cpool.tile([BL, OUT_W], F32, tag="out_tile")
        scr8 = out_tile[0:BL, CUM0 : CUM0 + RNN]  # never reaches DRAM from here

        wbf_drams = {
            1: nc.dram_tensor("w1bf_d", [BL, T], BF16),
            2: nc.dram_tensor("w2bf_d", [BL, T], BF16),
        }

        # ---------------- query projections (pq, spq) ----------------
        def project_query(wT, name):
            ps = ppm.tile([128, 512], F32, tag="pm", name="pq_ps")
            for rc in range(8):
                nc.tensor.matmul(
                    ps[:, 0:BL],
                    lhsT=wT[:, rc * 128 : (rc + 1) * 128],
                    rhs=qT[:, rc * BL : (rc + 1) * BL],
                    start=(rc == 0),
                    stop=(rc == 7),
                )
            sb = cpool.tile([128, BL], F32, tag=name)
            nc.vector.tensor_scalar_add(sb[:], ps[:, 0:BL], 0.0)
            return sb

        pq_sb = project_query(qWT, "pq_sb")
        smWT = sqWT = spq_sb = None  # created after the first ph1 row issues

        # ---------------- per-phase pieces ----------------
        def row_matmuls(memTs, wT, pq, v, with_loc, e_segs, b):
            """conv (optional) + pm (fp8 DoubleRow) + tanh + e-matmul for row b."""
            memT, roff = pair_view(memTs, b)
            wT3 = wT[:].rearrange("p (g m) -> p g m", g=4)  # 4 k-tiles of 128
            if with_loc:
                cps = ppm.tile([128, 512], F32, tag="pm", name="cps")
                conv_s = cspool.tile([NF, T], BF16, tag="convs", name="conv_s")
                for si, (t0, tl) in enumerate(SEGS):
                    nc.tensor.matmul(
                        cps[0:NF, 0:tl],
                        lhsT=convWT[:],
                        rhs=im2[:, b * T + t0 : b * T + t0 + tl],
                        start=True,
                        stop=True,
                        skip_group_check=True,
                    )
                    # split the psum->sbuf bounce across ACT and DVE so
                    # neither paces the row stream
                    if si == 0:
                        nc.scalar.activation(
                            conv_s[:, t0 : t0 + tl], cps[0:NF, 0:tl], AF.Copy
                        )
                    else:
                        nc.vector.tensor_scalar_add(
                            conv_s[:, t0 : t0 + tl], cps[0:NF, 0:tl], 0.0
                        )
            th = tpool.tile([128, T], BF16, tag="tanh", name="th")
            for si, (t0, tl) in enumerate(SEGS):
                pm = ppm.tile([128, 512], F32, tag="pm", name="pm")
                for g in range(2):  # two DoubleRow matmuls cover 4 k-tiles
                    sl = memT[:, roff + 2 * g * T + t0 : roff + 2 * g * T + t0 + tl]
                    rhs = _dc.replace(sl, ap=[sl.ap[0], [T, 2], [1, tl]])
                    nc.tensor.matmul(
                        pm[:, 0:tl],
                        lhsT=wT3[:, 2 * g : 2 * g + 2, :],
                        rhs=rhs,
                        start=(g == 0),
                        stop=(g == 1 and not with_loc),
                        perf_mode=PM_DR,
                        skip_group_check=True,
                    )
                if with_loc:
                    nc.tensor.matmul(
                        pm[:, 0:tl],
                        lhsT=ldWT[:],
                        rhs=conv_s[:, t0 : t0 + tl],
                        start=False,
                        stop=True,
                        skip_group_check=True,
                    )
                nc.scalar.activation(
                    th[:, t0 : t0 + tl],
                    pm[:, 0:tl],
                    AF.Tanh,
                    bias=pq[:, b : b + 1],
                    scale=1.0 / WS,
                )
                nc.tensor.matmul(
                    e_segs[si][0:BL, 0:tl],
                    lhsT=v[:, b * BL : (b + 1) * BL],
                    rhs=th[:, t0 : t0 + tl],
                    start=(b == 0),
                    stop=(b == BL - 1),
                    skip_group_check=True,
                )

        def make_esegs(name):
            return [
                ppe.tile([BL, 512], F32, tag=f"pe{si}", name=f"{name}{si}")
                for si in range(2)
            ]

        def phase_chain(e_segs, with_loc, w_rt, w_bf, w_dram, sig_rt):
            """sigmoid + normalization (+ alpha recurrence for phase 1)."""
            for si, (t0, tl) in enumerate(SEGS):
                nc.scalar.activation(
                    sig_rt[:, t0 : t0 + tl],
                    e_segs[si][0:BL, 0:tl],
                    AF.Sigmoid,
                    accum_out=cols[:, 6 + si : 7 + si],
                )
            if with_loc:
                nc.vector.tensor_add(cols[:, 8:9], cols[:, 6:7], cols[:, 7:8])
                nc.vector.reciprocal(cols[:, 9:10], cols[:, 8:9])
                # cum_new = awc + sig/sum(sig) in one fused op
                anew_rt = rtp.tile([BL, T], F32, tag="anew_rt", name="anew_rt")
                nc.vector.scalar_tensor_tensor(
                    out=anew_rt[:],
                    in0=sig_rt[:],
                    scalar=cols[:, 9:10],
                    in1=awc_rt[:],
                    op0=ALU.mult,
                    op1=ALU.add,
                )
                nc.sync.dma_start(out=out_d[:, CUM0 : CUM0 + T], in_=anew_rt[:])
                # monotonic alpha recurrence; the sigmoid-normalizing scalar
                # cancels in alignments = x/sum(x), so run the chain off raw
                # sig:  base = (1-u)*alpha + u*shift(alpha)
                #       w    = (base + 1e-8)*sig, normalized
                nc.vector.tensor_scalar(
                    out=cols[:, 1:2],
                    in0=cols[:, 0:1],
                    scalar1=-1.0,
                    scalar2=1.0,
                    op0=ALU.mult,
                    op1=ALU.add,
                )
                shift_rt = rtp.tile([BL, T], F32, tag="shift_rt", name="shift_rt")
                base_rt = rtp.tile([BL, T], F32, tag="base_rt", name="base_rt")
                nc.vector.memset(shift_rt[:, 0:1], 0.0)
                nc.vector.tensor_scalar_mul(
                    shift_rt[:, 1:T], alpha_rt[:, 0 : T - 1], cols[:, 0:1]
                )
                nc.vector.scalar_tensor_tensor(
                    out=base_rt[:],
                    in0=alpha_rt[:],
                    scalar=cols[:, 1:2],
                    in1=shift_rt[:],
                    op0=ALU.mult,
                    op1=ALU.add,
                )
                nc.vector.scalar_tensor_tensor(
                    out=base_rt[:],
                    in0=base_rt[:],
                    scalar=1e-8,
                    in1=sig_rt[:],
                    op0=ALU.add,
                    op1=ALU.mult,
                    accum_out=cols[:, 2:3],
                )
                nc.vector.reciprocal(cols[:, 3:4], cols[:, 2:3])
                nc.vector.tensor_scalar_mul(w_rt[:], base_rt[:], cols[:, 3:4])
            else:
                nc.vector.tensor_add(cols[:, 8:9], cols[:, 6:7], cols[:, 7:8])
                nc.vector.reciprocal(cols[:, 9:10], cols[:, 8:9])
                nc.vector.tensor_scalar_mul(w_rt[:], sig_rt[:], cols[:, 9:10])
            nc.scalar.activation(w_bf[:], w_rt[:], AF.Copy)
            nc.sync.dma_start(out=w_dram[:], in_=w_bf[:])

        def row_ctx(memTs, w_dram, ctxT, b):
            """bc broadcast matmul + 4 full-T weighted-sum accumulations.

            DVE rows: scalar_tensor_tensor with f32 accum (verified 2e-6).
            Pool rows: gpsimd tensor_tensor f8 x bf16 -> f32 product, then
            ACT Copy with accum_out (f32 accumulation follows the f32 input;
            gpsimd STT is rejected by codegen and ACT accumulation over a
            bf16 input is only ~1e-1 accurate).
            bc is bounced to bf16 SBUF for everyone: gpsimd cannot read
            PSUM, and the f8 x f32 STT path accumulates poorly.
            """
            memT, roff = pair_view(memTs, b)
            # broadcast w row b across 128 partitions with a step-0 DMA from
            # the DRAM copy of w_bf (the DMA engines are idle post-load, and
            # this frees ACT/PE/PSUM vs the old matmul+copy broadcast)
            bc_sb = bsbpool.tile([128, T], BF16, tag="bcsb", name="bc_sb")
            eng_dma = nc.sync if b % 2 == 0 else nc.gpsimd
            eng_dma.dma_start(
                out=bc_sb[:],
                in_=_dc.replace(w_dram[:], ap=[[0, 128], [1, T]], offset=b * T),
            )
            for c in range(4):
                cc = c * BL + b
                if b in DVE_ROWS:
                    scr = scpool_v.tile([128, T], BF16, tag="scr", name="scr")
                    nc.vector.scalar_tensor_tensor(
                        out=scr[:],
                        in0=memT[:, roff + c * T : roff + (c + 1) * T],
                        scalar=1.0,
                        in1=bc_sb[:],
                        op0=ALU.mult,
                        op1=ALU.mult,
                        accum_out=ctxT[:, cc : cc + 1],
                    )
                else:
                    scr32 = scpool_p.tile([128, T], F32, tag="scr32", name="scr32")
                    nc.gpsimd.tensor_tensor(
                        scr32[:],
                        memT[:, roff + c * T : roff + (c + 1) * T],
                        bc_sb[:],
                        ALU.mult,
                    )
                    dump = scpool_a.tile([128, T], BF16, tag="dump", name="dump")
                    nc.scalar.activation(
                        dump[:], scr32[:], AF.Copy, accum_out=ctxT[:, cc : cc + 1]
                    )

        def ctx_finalize(ctxT, ctx_off):
            # ctxT col (c, b) holds ctx[b, e] for e = 4p + c (interleaved
            # memory layout) -> transpose chunk c lands in out cols c::4
            for c in range(4):
                tp = ppm.tile([128, 512], F32, tag="pm", name="ctp")
                nc.tensor.transpose(
                    tp[0:BL, 0:128], ctxT[:, c * BL : (c + 1) * BL], ident[:]
                )
                sl = out_tile[0:BL, ctx_off + c : ctx_off + c + 1]
                dst = _dc.replace(sl, ap=[sl.ap[0], [4, 128]])
                nc.scalar.activation(dst, tp[0:BL, 0:128], AF.Copy)

        for _rep in range(repeat):
            if _rep > 0:
                memTs_p1 = {p: load_pair(mem_d, p, nc.sync) for p in range(BL // 2)}
                memTs_p2 = {p: load_pair(smem_d, p, nc.gpsimd) for p in range(BL // 2)}

            # ---- phase 1: location-sensitive monotonic attention ----
            e1 = make_esegs("e1")
            for b in range(BL):
                row_matmuls(memTs_p1, mWT, pq_sb, vmat, True, e1, b)
                if b == 1 and smWT is None:
                    # slot the phase-2 weight prep into the row stream's PE
                    # slack (rows are ACT/DVE-paced)
                    smWT = cpool.tile([128, EL], F8, tag="smWT")
                    transpose_into(smWT, smW_nat, EL, BF16, nc.scalar, interleave=True)
                    sqWT = cpool.tile([128, RNN], BF16, tag="sqWT")
                    transpose_into(sqWT, sqW_nat, RNN, BF16, nc.vector)
                    spq_sb = project_query(sqWT, "spq_sb")
            sig1 = rtp.tile([BL, T], F32, tag="sig1", name="sig1")
            w1 = rtp.tile([BL, T], F32, tag="w1", name="w1")
            w1_bf = rtp.tile([BL, T], BF16, tag="w1bf", name="w1_bf")
            phase_chain(e1, True, w1, w1_bf, wbf_drams[1], sig1)
            nc.sync.dma_start(out=out_d[:, ALIGN0 : ALIGN0 + T], in_=w1[:])

            ctxT1 = cpool.tile([128, 4 * BL], F32, tag="ctxT1")
            e2 = make_esegs("e2")
            # interleave phase-2 row matmuls with phase-1 ctx so the PE queue
            # never parks behind a bc matmul waiting on DVE/Pool
            for b in range(BL):
                row_matmuls(memTs_p2, smWT, spq_sb, svmat, False, e2, b)
                row_ctx(memTs_p1, wbf_drams[1], ctxT1, b)
            ctx_finalize(ctxT1, CTX0)

            # u_new = sigmoid([context, query] @ ta_W.T + ta_b)
            nc.vector.scalar_tensor_tensor(
                out=scr8[:, 0:E],
                in0=out_tile[0:BL, CTX0 : CTX0 + E],
                scalar=1.0,
                in1=taWb[:, 0:E],
                op0=ALU.mult,
                op1=ALU.mult,
                accum_out=cols[:, 4:5],
            )
            nc.vector.scalar_tensor_tensor(
                out=scr8[:, 0:RNN],
                in0=q_sb[:],
                scalar=1.0,
                in1=taWb[:, E : E + RNN],
                op0=ALU.mult,
                op1=ALU.mult,
                accum_out=cols[:, 10:11],
            )
            nc.vector.tensor_add(cols[:, 11:12], cols[:, 4:5], cols[:, 10:11])
            nc.scalar.activation(
                out_tile[0:BL, UN0 : UN0 + 1],
                cols[:, 11:12],
                AF.Sigmoid,
                bias=cols[:, 5:6],
            )

            # ---- phase 2: additive self-attention ----
            sig2 = rtp.tile([BL, T], F32, tag="sig2", name="sig2")
            w2 = rtp.tile([BL, T], F32, tag="w2", name="w2")
            w2_bf = rtp.tile([BL, T], BF16, tag="w2bf", name="w2_bf")
            phase_chain(e2, False, w2, w2_bf, wbf_drams[2], sig2)
            nc.sync.dma_start(out=out_d[:, W2_0 : W2_0 + T], in_=w2[:])

            ctxT2 = cpool.tile([128, 4 * BL], F32, tag="ctxT2")
            for b in range(BL):
                row_ctx(memTs_p2, wbf_drams[2], ctxT2, b)
            ctx_finalize(ctxT2, CTX2_0)

            nc.sync.dma_start(
                out=out_d[:, CTX0 : CTX0 + E], in_=out_tile[:, CTX0 : CTX0 + E]
            )
            nc.sync.dma_start(
                out=out_d[:, UN0 : UN0 + 1], in_=out_tile[:, UN0 : UN0 + 1]
            )
            nc.sync.dma_start(
                out=out_d[:, CTX2_0 : CTX2_0 + EL],
                in_=out_tile[:, CTX2_0 : CTX2_0 + EL],
            )

    if finalize:
        nc.finalize()
    return nc


_NC = None
RUN_KWARGS: dict = {}   # test harness can set {"trace": True}
LAST_RESULT = None      # BassKernelResults of the most recent kernel() call


def _get_nc():
    global _NC
    if _NC is None:
        _NC = build_nc()
    return _NC


def make_in_map(shard: dict) -> dict:
    """Device in_map for ONE core's shard (keys as in setup_inputs)."""
    f = lambda k: np.ascontiguousarray(np.asarray(shard[k], dtype=np.float32))
    bf = ml_dtypes.bfloat16
    f8 = ml_dtypes.float8_e4m3
    return {
        "query": f("query"),
        "memory": np.ascontiguousarray(f("memory").transpose(0, 2, 1).astype(f8)),
        "self_memory": np.ascontiguousarray(
            f("self_memory").transpose(0, 2, 1).astype(f8)
        ),
        "attention_weights": f("attention_weights").astype(bf),
        "attention_weights_cum": f("attention_weights_cum"),
        "awc_bf": f("attention_weights_cum").astype(bf),
        "alpha": f("alpha"),
        "u": f("u"),
        "memory_W": (f("memory_W") * WS).astype(bf),
        "query_W": f("query_W").astype(bf),
        "v_W": f("v_W"),
        "loc_conv_W": f("loc_conv_W"),
        "loc_dense_W": f("loc_dense_W"),
        "ta_W": f("ta_W"),
        "ta_b": f("ta_b").reshape(1, 1),
        "self_memory_W": (f("self_memory_W") * WS).astype(bf),
        "self_query_W": f("self_query_W").astype(bf),
        "self_v_W": f("self_v_W"),
    }


def kernel(**inputs) -> np.ndarray:
    f = lambda k: np.ascontiguousarray(np.asarray(inputs[k], dtype=np.float32))
    bf = ml_dtypes.bfloat16
    f8 = ml_dtypes.float8_e4m3
    rep = {
        "memory_W": (f("memory_W") * WS).astype(bf),
        "query_W": f("query_W").astype(bf),
        "v_W": f("v_W"),
        "loc_conv_W": f("loc_conv_W"),
        "loc_dense_W": f("loc_dense_W"),
        "ta_W": f("ta_W"),
        "ta_b": f("ta_b").reshape(1, 1),
        "self_memory_W": (f("self_memory_W") * WS).astype(bf),
        "self_query_W": f("self_query_W").astype(bf),
        "self_v_W": f("self_v_W"),
    }
    mem_t = np.ascontiguousarray(f("memory").transpose(0, 2, 1).astype(f8))
    smem_t = np.ascontiguousarray(f("self_memory").transpose(0, 2, 1).astype(f8))
    aw_bf = f("attention_weights").astype(bf)
    awc = f("attention_weights_cum")
    awc_bf = awc.astype(bf)
    q = f("query")
    alpha = f("alpha")
    u = f("u")
    in_maps = []
    for i in range(NCORES):
        sl = slice(i * BL, (i + 1) * BL)
        m = dict(rep)
        m["query"] = q[sl]
        m["memory"] = mem_t[sl]
        m["self_memory"] = smem_t[sl]
        m["attention_weights"] = aw_bf[sl]
        m["attention_weights_cum"] = awc[sl]
        m["awc_bf"] = awc_bf[sl]
        m["alpha"] = alpha[sl]
        m["u"] = u[sl]
        in_maps.append(m)
    global LAST_RESULT
    res = run_bass_kernel_spmd(
        _get_nc(), in_maps, core_ids=list(range(NCORES)), **RUN_KWARGS
    )
    LAST_RESULT = res
    return np.concatenate([res.results[i]["out"] for i in range(NCORES)], axis=0)


# revision 44
# speedup vs baseline: 4.4525x; 1.2043x over previous
"""Trainium2 Bass kernel for nn_Attention_15410342658774 (v2).

Location-sensitive monotonic attention + additive self-attention
(Tacotron-style), B=64, T=1000, E=EL=512, RNN=1024, AD=128.

Pure data parallel across 8 NeuronCores (8 batch rows each, weights
replicated).  Host pre-transposes `memory`/`self_memory` to [B, E, T]
and casts to fp8e4 (halves the HBM stream vs bf16 and enables DoubleRow
matmuls); weight matrices ride along as fp8e4 scaled by 64 (folded back
out via the tanh activation's input scale).

v2 changes vs the 162us baseline (cost-model-driven):
  - fp8e4 memory stream: DMA_ENGINES floor 56us -> ~28us; DoubleRow
    fp8 matmuls for pm (2 k-tiles per instruction at 0.5 cyc/row).
  - All big DMAs on dedicated queues (sync for phase 1 + weights,
    gpsimd for phase 2 + im2col) - never on the ACT/DVE/PE queues,
    since a dma_start occupies its queue for the whole transfer.
  - ctx accumulation (the 82us DVE hotspot; scalar_tensor_tensor has
    no DVE fast mode) split DVE/Pool: DVE rows read the bc broadcast
    straight from PSUM, Pool rows get an ACT-copied bf16 SBUF view
    (gpsimd cannot touch PSUM).
  - bc broadcast matmuls in bf16 (they were f32 = 4 cyc/row on PE).
  - reductions fused into producers via accum_out (sigmoid-sum,
    alpha-chain sum).
  - PSUM budget reworked to exactly 8 banks: pm 2 + bc 2x2 + e-segs 2.

Hardware constraints baked in (from the v1 session + cost model):
  - matmul operands/outputs must start at partition 0/32/64; PSUM
    matmul outputs must not cross a 2KB bank.
  - DMA access patterns: at most 3 [step,count] dims, innermost step 1.
  - TENSOR_TENSOR_REDUCE crashes the exec unit; scalar_tensor_tensor's
    accum_out is the working per-partition reduction.
  - gpsimd (Pool) engine: SBUF only, no PSUM access.
"""

import dataclasses as _dc
import sys

import numpy as np

_TRN = "/opt/trn_rl_repo"
if _TRN not in sys.path:
    sys.path.insert(0, _TRN)

from contextlib import ExitStack

import ml_dtypes

import concourse.bacc as bacc
import concourse.bass as bass
import concourse.mybir as mybir
from concourse.bass_utils import run_bass_kernel_spmd
from concourse.masks import make_identity
from concourse.tile import TileContext

B, T = 64, 1000
E, EL, RNN, AD = 512, 512, 1024, 128
NF, K = 32, 31
PAD = (K - 1) // 2
NCORES = 8
BL = B // NCORES  # 8 batch rows per core
F32 = mybir.dt.float32
BF16 = mybir.dt.bfloat16
F8 = mybir.dt.float8e4
AF = mybir.ActivationFunctionType
ALU = mybir.AluOpType
AX = mybir.AxisListType
PM_DR = mybir.MatmulPerfMode.DoubleRow
SEGS = [(0, 512), (512, 488)]  # T split at the 512-float PSUM bank boundary
WS = 64.0  # fp8 weight pre-scale (exact power of two)

# output packing: [context(E) | alignments(T) | u_new(1) | cum_new(T) | ctx2(EL) | w2(T)]
CTX0 = 0
ALIGN0 = E
UN0 = E + T
CUM0 = E + T + 1
CTX2_0 = E + 2 * T + 1
W2_0 = E + EL + 2 * T + 1
OUT_W = E + EL + 3 * T + 1  # 4025

DVE_ROWS = (0, 1, 2, 3, 4, 5, 6, 7)  # all ctx rows on DVE (Pool TT was slow on HW)


def build_nc(finalize: bool = True, repeat: int = 1) -> bass.Bass:
    nc = bacc.Bacc()

    q_d = nc.declare_dram_parameter("query", [BL, RNN], F32, isOutput=False)
    # pre-transposed [E, T] per row, fp8e4 (host-prepared)
    mem_d = nc.declare_dram_parameter("memory", [BL, E, T], F8, isOutput=False)
    smem_d = nc.declare_dram_parameter("self_memory", [BL, EL, T], F8, isOutput=False)
    aw_d = nc.declare_dram_parameter("attention_weights", [BL, T], BF16, isOutput=False)
    awc_d = nc.declare_dram_parameter(
        "attention_weights_cum", [BL, T], F32, isOutput=False
    )
    awcb_d = nc.declare_dram_parameter("awc_bf", [BL, T], BF16, isOutput=False)
    al_d = nc.declare_dram_parameter("alpha", [BL, T], F32, isOutput=False)
    u_d = nc.declare_dram_parameter("u", [BL, 1], F32, isOutput=False)
    mW_d = nc.declare_dram_parameter("memory_W", [AD, E], BF16, isOutput=False)  # x64
    qW_d = nc.declare_dram_parameter("query_W", [AD, RNN], BF16, isOutput=False)
    vW_d = nc.declare_dram_parameter("v_W", [1, AD], F32, isOutput=False)
    cW_d = nc.declare_dram_parameter("loc_conv_W", [NF, 2, K], F32, isOutput=False)
    dW_d = nc.declare_dram_parameter("loc_dense_W", [AD, NF], F32, isOutput=False)
    taW_d = nc.declare_dram_parameter("ta_W", [1, E + RNN], F32, isOutput=False)
    tab_d = nc.declare_dram_parameter("ta_b", [1, 1], F32, isOutput=False)
    smW_d = nc.declare_dram_parameter("self_memory_W", [AD, EL], BF16, isOutput=False)
    sqW_d = nc.declare_dram_parameter("self_query_W", [AD, RNN], BF16, isOutput=False)
    svW_d = nc.declare_dram_parameter("self_v_W", [1, AD], F32, isOutput=False)
    out_d = nc.declare_dram_parameter("out", [BL, OUT_W], F32, isOutput=True)

    with ExitStack() as ctx:
        tc = ctx.enter_context(TileContext(nc))
        cpool = ctx.enter_context(tc.tile_pool(name="const", bufs=1))
        mpool = ctx.enter_context(tc.tile_pool(name="mem", bufs=8))
        tpool = ctx.enter_context(tc.tile_pool(name="tanhp", bufs=3))
        cspool = ctx.enter_context(tc.tile_pool(name="convsp", bufs=2))
        bsbpool = ctx.enter_context(tc.tile_pool(name="bsb", bufs=3))
        # separate STT scratch pools per engine — a shared pool would make
        # every row's scratch reuse the other engine's slots, serializing the
        # DVE and Pool ctx streams into lockstep
        scpool_v = ctx.enter_context(tc.tile_pool(name="scrv", bufs=2))
        scpool_p = ctx.enter_context(tc.tile_pool(name="scrp", bufs=2))
        scpool_a = ctx.enter_context(tc.tile_pool(name="scra", bufs=2))
        rtp = ctx.enter_context(tc.tile_pool(name="rtp", bufs=1))
        ldp = ctx.enter_context(tc.tile_pool(name="ldp", bufs=2))
        # PSUM: 8 banks of [128, 512] f32. pm/conv 4 (2 banks spare) + e-segs 2.
        ppm = ctx.enter_context(tc.tile_pool(name="ppm", bufs=4, space="PSUM"))
        ppe = ctx.enter_context(tc.tile_pool(name="ppe", bufs=1, space="PSUM"))

        # identity goes FIRST on the gpsimd queue — everything transposes
        # through it, and gpsimd also carries the phase-2 DMA stream
        ident = cpool.tile([128, 128], F32, tag="ident")
        make_identity(nc, ident[:])
        ident_bf = cpool.tile([128, 128], BF16, tag="ident_bf")
        nc.scalar.activation(ident_bf[:], ident[:], AF.Copy)

        # ---------------- DMA issue: weights + phase-1 memory on sync,
        # ---------------- im2col + phase-2 memory on gpsimd ----------------
        # Memory rides in the "(p c) t" interleaved layout: partition p holds
        # E rows 4p..4p+3 as one contiguous 4000B run per row, so one DMA can
        # carry TWO batch rows in a 3-dim access pattern (the per-DMA fixed
        # cost - dge + semaphore - was pacing each queue to one 0.5MB tile
        # per ~3us).  E row 4p+c lives at col (b%2)*4T + c*T.
        def load_pair(mem_dram, pair, eng):
            memT = mpool.tile([128, 8 * T], F8, tag="memT", name="memT")
            src = _dc.replace(
                mem_dram[:],
                ap=[[4 * T, 128], [E * T, 2], [1, 4 * T]],
                offset=pair * 2 * E * T,
            )
            eng.dma_start(
                out=memT[:].rearrange("p (b x) -> p b x", b=2),
                in_=src,
            )
            return memT

        def pair_view(memTs, b):
            return memTs[b // 2], (b % 2) * 4 * T

        smW_nat = ldp.tile([AD, EL], BF16, tag="smw_nat", name="smW_nat")
        nc.sync.dma_start(out=smW_nat[:], in_=smW_d[:])
        memTs_p2 = {0: load_pair(smem_d, 0, nc.sync)}
        sqW_nat = ldp.tile([AD, RNN], BF16, tag="sqw_nat", name="sqW_nat")
        nc.sync.dma_start(out=sqW_nat[:], in_=sqW_d[:])
        q_sb = cpool.tile([BL, RNN], F32, tag="q_sb")
        nc.sync.dma_start(out=q_sb[:], in_=q_d[:])
        memTs_p2[1] = load_pair(smem_d, 1, nc.sync)
        v_sb = cpool.tile([128, 2], F32, tag="v_sb")
        nc.sync.dma_start(out=v_sb[:, 1:2], in_=svW_d[:].rearrange("o a -> a o"))
        nc.sync.dma_start(out=v_sb[:, 0:1], in_=vW_d[:].rearrange("o a -> a o"))
        mW_nat = ldp.tile([AD, E], BF16, tag="mw_nat", name="mW_nat")
        nc.sync.dma_start(out=mW_nat[:], in_=mW_d[:])
        qW_nat = ldp.tile([AD, RNN], BF16, tag="qw_nat", name="qW_nat")
        nc.sync.dma_start(out=qW_nat[:], in_=qW_d[:])
        memTs_p2[2] = load_pair(smem_d, 2, nc.sync)

        convWT_f = cpool.tile([2 * K, NF], F32, tag="convWT_f")  # [(c k), o]
        nc.sync.dma_start(out=convWT_f[:], in_=cW_d[:].rearrange("o c k -> (c k) o"))
        ldWT_f = cpool.tile([NF, AD], F32, tag="ldWT_f")
        nc.sync.dma_start(out=ldWT_f[:], in_=dW_d[:].rearrange("a f -> f a"))
        # per-row scalars: 0=u 1=1-u 2..3 alpha-sums 4=s_ta1 5=ta_b 6..7 sig-sums
        cols = cpool.tile([BL, 12], F32, tag="cols")
        nc.sync.dma_start(
            out=cols[:, 5:6], in_=_dc.replace(tab_d[:], ap=[[0, BL], [1, 1]])
        )
        awc_rt = rtp.tile([BL, T], F32, tag="awc_rt", name="awc_rt")
        nc.sync.dma_start(out=awc_rt[:], in_=awc_d[:])
        memTs_p2[3] = load_pair(smem_d, 3, nc.sync)

        taWb = cpool.tile([BL, E + RNN], F32, tag="taWb")
        nc.sync.dma_start(
            out=taWb[:], in_=_dc.replace(taW_d[:], ap=[[0, BL], [1, E + RNN]])
        )
        alpha_rt = rtp.tile([BL, T], F32, tag="alpha_rt", name="alpha_rt")
        nc.sync.dma_start(out=alpha_rt[:], in_=al_d[:])
        nc.sync.dma_start(out=cols[:, 0:1], in_=u_d[:])

        # im2col of [attention_weights; attention_weights_cum] for the
        # location conv: rows (c, k), cols (b, t); zero-padded edges via a
        # DRAM bounce so the sliding window is 2 DMAs.  All on gpsimd.
        TP = T + 2 * PAD
        pad_d = nc.dram_tensor("awc_pad", [2 * BL, TP], BF16)
        zero_sb = cpool.tile([2 * BL, 16], BF16, tag="zero_sb")
        nc.vector.memset(zero_sb[:], 0.0)
        nc.gpsimd.dma_start(out=pad_d[:, 0:PAD], in_=zero_sb[:, 0:PAD])
        nc.gpsimd.dma_start(out=pad_d[:, TP - PAD : TP], in_=zero_sb[:, 0:PAD])
        nc.gpsimd.dma_start(out=pad_d[0:BL, PAD : PAD + T], in_=aw_d[:])
        nc.gpsimd.dma_start(out=pad_d[BL : 2 * BL, PAD : PAD + T], in_=awcb_d[:])
        im2 = cpool.tile([2 * K, BL * T], BF16, tag="im2")
        for c in range(2):
            src = _dc.replace(
                pad_d[:], ap=[[1, K], [TP, BL], [1, T]], offset=c * BL * TP
            )
            nc.gpsimd.dma_start(
                out=im2[c * K : (c + 1) * K, :].rearrange("k (b t) -> k b t", b=BL),
                in_=src,
            )
        memTs_p1 = {pr: load_pair(mem_d, pr, nc.gpsimd) for pr in range(BL // 2)}

        # ---------------- constants / on-chip weight prep ----------------
        # setup copies run on DVE (idle until the phase-1 chain) so the ACT
        # queue reaches conv_s/tanh — and thus sigmoid1 — as early as possible
        def transpose_into(dst, nat, ncols, dt, eng_copy, interleave=False):
            idn = {F32: ident, BF16: ident_bf}[dt]
            for c in range(ncols // 128):
                if interleave:
                    # column set {4j + c}: gives W.T rows for the "(p c) t"
                    # memory layout where E row 4p+c sits at partition p
                    sl = nat[:, c : c + 1]
                    src = _dc.replace(sl, ap=[sl.ap[0], [4, 128]])
                else:
                    src = nat[:, c * 128 : (c + 1) * 128]
                tp = ppm.tile([128, 512], dt, tag="pm", name="wtp")
                nc.tensor.transpose(tp[:, 0:128], src, idn[:])
                if eng_copy is nc.scalar:
                    nc.scalar.activation(
                        dst[:, c * 128 : (c + 1) * 128], tp[:, 0:128], AF.Copy
                    )
                else:
                    eng_copy.tensor_scalar_add(
                        dst[:, c * 128 : (c + 1) * 128], tp[:, 0:128], 0.0
                    )

        smWT = cpool.tile([128, EL], F8, tag="smWT")
        transpose_into(smWT, smW_nat, EL, BF16, nc.scalar, interleave=True)
        sqWT = cpool.tile([128, RNN], BF16, tag="sqWT")
        transpose_into(sqWT, sqW_nat, RNN, BF16, nc.vector)
        # mWT/qWT/pq are deferred into the phase loop (issued after the first
        # phase-2 rows) so their transposes don't delay the first pm

        qT = cpool.tile([128, 8 * BL], BF16, tag="qT")  # cols (rchunk, b)
        for c in range(8):
            tp = ppm.tile([128, 512], F32, tag="pm", name="qtp")
            nc.tensor.transpose(
                tp[:, 0:BL], q_sb[:, c * 128 : (c + 1) * 128], ident[0:BL, 0:BL]
            )
            nc.vector.tensor_scalar_add(qT[:, c * BL : (c + 1) * BL], tp[:, 0:BL], 0.0)

        convWT = cpool.tile([2 * K, NF], BF16, tag="convWT")
        nc.vector.tensor_scalar_add(convWT[:], convWT_f[:], 0.0)
        ldWT = cpool.tile([NF, AD], BF16, tag="ldWT")  # x64 to match fp8 weight scale
        nc.vector.tensor_scalar_mul(ldWT[:], ldWT_f[:], WS)

        # vmat[:, bi*BL + j] = v if j == bi else 0 — row-masked v so the
        # e contraction for row bi lands in PSUM partition bi.
        def masked_v(col, name):
            t = cpool.tile([128, BL * BL], BF16, tag=name)
            nc.vector.memset(t[:], 0.0)
            for bi in range(BL):
                c = bi * BL + bi
                nc.vector.tensor_scalar_add(
                    t[:, c : c + 1], v_sb[:, col : col + 1], 0.0
                )
            return t

        vmat = masked_v(0, "vmat")
        svmat = masked_v(1, "svmat")

        # context / u_new staging for the whole local batch (partitions 0..BL)
        out_tile = cpool.tile([BL, OUT_W], F32, tag="out_tile")
        scr8 = out_tile[0:BL, CUM0 : CUM0 + RNN]  # never reaches DRAM from here

        wbf_drams = {
            1: nc.dram_tensor("w1bf_d", [BL, T], BF16),
            2: nc.dram_tensor("w2bf_d", [BL, T], BF16),
        }

        # ---------------- query projections (pq, spq) ----------------
        def project_query(wT, name):
            ps = ppm.tile([128, 512], F32, tag="pm", name="pq_ps")
            for rc in range(8):
                nc.tensor.matmul(
                    ps[:, 0:BL],
                    lhsT=wT[:, rc * 128 : (rc + 1) * 128],
                    rhs=qT[:, rc * BL : (rc + 1) * BL],
                    start=(rc == 0),
                    stop=(rc == 7),
                )
            sb = cpool.tile([128, BL], F32, tag=name)
            nc.vector.tensor_scalar_add(sb[:], ps[:, 0:BL], 0.0)
            return sb

        spq_sb = project_query(sqWT, "spq_sb")
        mWT = qWT = pq_sb = None  # created after the first ph2 row issues

        # ---------------- per-phase pieces ----------------
        def row_matmuls(memTs, wT, pq, v, with_loc, e_segs, b):
            """conv (optional) + pm (fp8 DoubleRow) + tanh + e-matmul for row b."""
            memT, roff = pair_view(memTs, b)
            wT3 = wT[:].rearrange("p (g m) -> p g m", g=4)  # 4 k-tiles of 128
            if with_loc:
                cps = ppm.tile([128, 512], F32, tag="pm", name="cps")
                conv_s = cspool.tile([NF, T], BF16, tag="convs", name="conv_s")
                for si, (t0, tl) in enumerate(SEGS):
                    nc.tensor.matmul(
                        cps[0:NF, 0:tl],
                        lhsT=convWT[:],
                        rhs=im2[:, b * T + t0 : b * T + t0 + tl],
                        start=True,
                        stop=True,
                        skip_group_check=True,
                    )
                    # split the psum->sbuf bounce across ACT and DVE so
                    # neither paces the row stream
                    if si == 0:
                        nc.scalar.activation(
                            conv_s[:, t0 : t0 + tl], cps[0:NF, 0:tl], AF.Copy
                        )
                    else:
                        nc.vector.tensor_scalar_add(
                            conv_s[:, t0 : t0 + tl], cps[0:NF, 0:tl], 0.0
                        )
            th = tpool.tile([128, T], BF16, tag="tanh", name="th")
            for si, (t0, tl) in enumerate(SEGS):
                pm = ppm.tile([128, 512], F32, tag="pm", name="pm")
                for g in range(2):  # two DoubleRow matmuls cover 4 k-tiles
                    sl = memT[:, roff + 2 * g * T + t0 : roff + 2 * g * T + t0 + tl]
                    rhs = _dc.replace(sl, ap=[sl.ap[0], [T, 2], [1, tl]])
                    nc.tensor.matmul(
                        pm[:, 0:tl],
                        lhsT=wT3[:, 2 * g : 2 * g + 2, :],
                        rhs=rhs,
                        start=(g == 0),
                        stop=(g == 1 and not with_loc),
                        perf_mode=PM_DR,
                        skip_group_check=True,
                    )
                if with_loc:
                    nc.tensor.matmul(
                        pm[:, 0:tl],
                        lhsT=ldWT[:],
                        rhs=conv_s[:, t0 : t0 + tl],
                        start=False,
                        stop=True,
                        skip_group_check=True,
                    )
                nc.scalar.activation(
                    th[:, t0 : t0 + tl],
                    pm[:, 0:tl],
                    AF.Tanh,
                    bias=pq[:, b : b + 1],
                    scale=1.0 / WS,
                )
                nc.tensor.matmul(
                    e_segs[si][0:BL, 0:tl],
                    lhsT=v[:, b * BL : (b + 1) * BL],
                    rhs=th[:, t0 : t0 + tl],
                    start=(b == 0),
                    stop=(b == BL - 1),
                    skip_group_check=True,
                )

        def make_esegs(name):
            return [
                ppe.tile([BL, 512], F32, tag=f"pe{si}", name=f"{name}{si}")
                for si in range(2)
            ]

        def phase_chain(e_segs, with_loc, w_rt, w_bf, w_dram, sig_rt):
            """sigmoid + normalization (+ alpha recurrence for phase 1)."""
            for si, (t0, tl) in enumerate(SEGS):
                nc.scalar.activation(
                    sig_rt[:, t0 : t0 + tl],
                    e_segs[si][0:BL, 0:tl],
                    AF.Sigmoid,
                    accum_out=cols[:, 6 + si : 7 + si],
                )
            if with_loc:
                nc.vector.tensor_add(cols[:, 8:9], cols[:, 6:7], cols[:, 7:8])
                nc.vector.reciprocal(cols[:, 9:10], cols[:, 8:9])
                # cum_new = awc + sig/sum(sig) in one fused op
                anew_rt = rtp.tile([BL, T], F32, tag="anew_rt", name="anew_rt")
                nc.vector.scalar_tensor_tensor(
                    out=anew_rt[:],
                    in0=sig_rt[:],
                    scalar=cols[:, 9:10],
                    in1=awc_rt[:],
                    op0=ALU.mult,
                    op1=ALU.add,
                )
                nc.sync.dma_start(out=out_d[:, CUM0 : CUM0 + T], in_=anew_rt[:])
                # monotonic alpha recurrence; the sigmoid-normalizing scalar
                # cancels in alignments = x/sum(x), so run the chain off raw
                # sig:  base = (1-u)*alpha + u*shift(alpha)
                #       w    = (base + 1e-8)*sig, normalized
                nc.vector.tensor_scalar(
                    out=cols[:, 1:2],
                    in0=cols[:, 0:1],
                    scalar1=-1.0,
                    scalar2=1.0,
                    op0=ALU.mult,
                    op1=ALU.add,
                )
                shift_rt = rtp.tile([BL, T], F32, tag="shift_rt", name="shift_rt")
                base_rt = rtp.tile([BL, T], F32, tag="base_rt", name="base_rt")
                nc.vector.memset(shift_rt[:, 0:1], 0.0)
                nc.vector.tensor_scalar_mul(
                    shift_rt[:, 1:T], alpha_rt[:, 0 : T - 1], cols[:, 0:1]
                )
                nc.vector.scalar_tensor_tensor(
                    out=base_rt[:],
                    in0=alpha_rt[:],
                    scalar=cols[:, 1:2],
                    in1=shift_rt[:],
                    op0=ALU.mult,
                    op1=ALU.add,
                )
                nc.vector.scalar_tensor_tensor(
                    out=base_rt[:],
                    in0=base_rt[:],
                    scalar=1e-8,
                    in1=sig_rt[:],
                    op0=ALU.add,
                    op1=ALU.mult,
                    accum_out=cols[:, 2:3],
                )
                nc.vector.reciprocal(cols[:, 3:4], cols[:, 2:3])
                nc.vector.tensor_scalar_mul(w_rt[:], base_rt[:], cols[:, 3:4])
            else:
                nc.vector.tensor_add(cols[:, 8:9], cols[:, 6:7], cols[:, 7:8])
                nc.vector.reciprocal(cols[:, 9:10], cols[:, 8:9])
                nc.vector.tensor_scalar_mul(w_rt[:], sig_rt[:], cols[:, 9:10])
            nc.scalar.activation(w_bf[:], w_rt[:], AF.Copy)
            nc.sync.dma_start(out=w_dram[:], in_=w_bf[:])

        def row_ctx(memTs, w_dram, ctxT, b):
            """bc broadcast matmul + 4 full-T weighted-sum accumulations.

            DVE rows: scalar_tensor_tensor with f32 accum (verified 2e-6).
            Pool rows: gpsimd tensor_tensor f8 x bf16 -> f32 product, then
            ACT Copy with accum_out (f32 accumulation follows the f32 input;
            gpsimd STT is rejected by codegen and ACT accumulation over a
            bf16 input is only ~1e-1 accurate).
            bc is bounced to bf16 SBUF for everyone: gpsimd cannot read
            PSUM, and the f8 x f32 STT path accumulates poorly.
            """
            memT, roff = pair_view(memTs, b)
            # broadcast w row b across 128 partitions with a step-0 DMA from
            # the DRAM copy of w_bf (the DMA engines are idle post-load, and
            # this frees ACT/PE/PSUM vs the old matmul+copy broadcast)
            bc_sb = bsbpool.tile([128, T], BF16, tag="bcsb", name="bc_sb")
            eng_dma = nc.sync if b % 2 == 0 else nc.gpsimd
            eng_dma.dma_start(
                out=bc_sb[:],
                in_=_dc.replace(w_dram[:], ap=[[0, 128], [1, T]], offset=b * T),
            )
            for c in range(4):
                cc = c * BL + b
                if b in DVE_ROWS:
                    scr = scpool_v.tile([128, T], BF16, tag="scr", name="scr")
                    nc.vector.scalar_tensor_tensor(
                        out=scr[:],
                        in0=memT[:, roff + c * T : roff + (c + 1) * T],
                        scalar=1.0,
                        in1=bc_sb[:],
                        op0=ALU.mult,
                        op1=ALU.mult,
                        accum_out=ctxT[:, cc : cc + 1],
                    )
                else:
                    scr32 = scpool_p.tile([128, T], F32, tag="scr32", name="scr32")
                    nc.gpsimd.tensor_tensor(
                        scr32[:],
                        memT[:, roff + c * T : roff + (c + 1) * T],
                        bc_sb[:],
                        ALU.mult,
                    )
                    dump = scpool_a.tile([128, T], BF16, tag="dump", name="dump")
                    nc.scalar.activation(
                        dump[:], scr32[:], AF.Copy, accum_out=ctxT[:, cc : cc + 1]
                    )

        def ctx_finalize(ctxT, ctx_off):
            # ctxT col (c, b) holds ctx[b, e] for e = 4p + c (interleaved
            # memory layout) -> transpose chunk c lands in out cols c::4
            for c in range(4):
                tp = ppm.tile([128, 512], F32, tag="pm", name="ctp")
                nc.tensor.transpose(
                    tp[0:BL, 0:128], ctxT[:, c * BL : (c + 1) * BL], ident[:]
                )
                sl = out_tile[0:BL, ctx_off + c : ctx_off + c + 1]
                dst = _dc.replace(sl, ap=[sl.ap[0], [4, 128]])
                nc.scalar.activation(dst, tp[0:BL, 0:128], AF.Copy)

        for _rep in range(repeat):
            if _rep > 0:
                memTs_p2 = {p: load_pair(smem_d, p, nc.sync) for p in range(BL // 2)}
                memTs_p1 = {p: load_pair(mem_d, p, nc.gpsimd) for p in range(BL // 2)}

            # ---- phase 2 (self-attention) FIRST: no conv dependency and a
            # ---- 3-op chain, so the DVE ctx stream starts ~15us earlier;
            # ---- phase 1's conv/alpha chain latency then hides inside it
            e2 = make_esegs("e2")
            for b in range(BL):
                row_matmuls(memTs_p2, smWT, spq_sb, svmat, False, e2, b)
                if b == 1 and mWT is None:
                    # slot the phase-1 weight prep into the row stream's PE
                    # slack (rows are ACT/DVE-paced)
                    mWT = cpool.tile([128, E], F8, tag="mWT")
                    transpose_into(mWT, mW_nat, E, BF16, nc.scalar, interleave=True)
                    qWT = cpool.tile([128, RNN], BF16, tag="qWT")
                    transpose_into(qWT, qW_nat, RNN, BF16, nc.vector)
                    pq_sb = project_query(qWT, "pq_sb")
            sig2 = rtp.tile([BL, T], F32, tag="sig2", name="sig2")
            w2 = rtp.tile([BL, T], F32, tag="w2", name="w2")
            w2_bf = rtp.tile([BL, T], BF16, tag="w2bf", name="w2_bf")
            phase_chain(e2, False, w2, w2_bf, wbf_drams[2], sig2)
            nc.sync.dma_start(out=out_d[:, W2_0 : W2_0 + T], in_=w2[:])

            ctxT2 = cpool.tile([128, 4 * BL], F32, tag="ctxT2")
            e1 = make_esegs("e1")
            # interleave phase-1 row matmuls with phase-2 ctx so the PE queue
            # stays dense while DVE chews the ctx stream
            for b in range(BL):
                row_matmuls(memTs_p1, mWT, pq_sb, vmat, True, e1, b)
                row_ctx(memTs_p2, wbf_drams[2], ctxT2, b)
            ctx_finalize(ctxT2, CTX2_0)

            # ---- phase 1: location-sensitive monotonic attention ----
            sig1 = rtp.tile([BL, T], F32, tag="sig1", name="sig1")
            w1 = rtp.tile([BL, T], F32, tag="w1", name="w1")
            w1_bf = rtp.tile([BL, T], BF16, tag="w1bf", name="w1_bf")
            phase_chain(e1, True, w1, w1_bf, wbf_drams[1], sig1)
            nc.sync.dma_start(out=out_d[:, ALIGN0 : ALIGN0 + T], in_=w1[:])

            ctxT1 = cpool.tile([128, 4 * BL], F32, tag="ctxT1")
            for b in range(BL):
                row_ctx(memTs_p1, wbf_drams[1], ctxT1, b)
            ctx_finalize(ctxT1, CTX0)

            # u_new = sigmoid([context, query] @ ta_W.T + ta_b)
            nc.vector.scalar_tensor_tensor(
                out=scr8[:, 0:E],
                in0=out_tile[0:BL, CTX0 : CTX0 + E],
                scalar=1.0,
                in1=taWb[:, 0:E],
                op0=ALU.mult,
                op1=ALU.mult,
                accum_out=cols[:, 4:5],
            )
            nc.vector.scalar_tensor_tensor(
                out=scr8[:, 0:RNN],
                in0=q_sb[:],
                scalar=1.0,
                in1=taWb[:, E : E + RNN],
                op0=ALU.mult,
                op1=ALU.mult,
                accum_out=cols[:, 10:11],
            )
            nc.vector.tensor_add(cols[:, 11:12], cols[:, 4:5], cols[:, 10:11])
            nc.scalar.activation(
                out_tile[0:BL, UN0 : UN0 + 1],
                cols[:, 11:12],
                AF.Sigmoid,
                bias=cols[:, 5:6],
            )

            nc.sync.dma_start(
                out=out_d[:, CTX0 : CTX0 + E], in_=out_tile[:, CTX0 : CTX0 + E]
            )
            nc.sync.dma_start(
                out=out_d[:, UN0 : UN0 + 1], in_=out_tile[:, UN0 : UN0 + 1]
            )
            nc.sync.dma_start(
                out=out_d[:, CTX2_0 : CTX2_0 + EL],
                in_=out_tile[:, CTX2_0 : CTX2_0 + EL],
            )

    if finalize:
        nc.finalize()
    return nc


_NC = None
RUN_KWARGS: dict = {}   # test harness can set {"trace": True}
LAST_RESULT = None      # BassKernelResults of the most recent kernel() call


def _get_nc():
    global _NC
    if _NC is None:
        _NC = build_nc()
    return _NC


def make_in_map(shard: dict) -> dict:
    """Device in_map for ONE core's shard (keys as in setup_inputs)."""
    f = lambda k: np.ascontiguousarray(np.asarray(shard[k], dtype=np.float32))
    bf = ml_dtypes.bfloat16
    f8 = ml_dtypes.float8_e4m3
    return {
        "query": f("query"),
        "memory": np.ascontiguousarray(f("memory").transpose(0, 2, 1).astype(f8)),
        "self_memory": np.ascontiguousarray(
            f("self_memory").transpose(0, 2, 1).astype(f8)
        ),
        "attention_weights": f("attention_weights").astype(bf),
        "attention_weights_cum": f("attention_weights_cum"),
        "awc_bf": f("attention_weights_cum").astype(bf),
        "alpha": f("alpha"),
        "u": f("u"),
        "memory_W": (f("memory_W") * WS).astype(bf),
        "query_W": f("query_W").astype(bf),
        "v_W": f("v_W"),
        "loc_conv_W": f("loc_conv_W"),
        "loc_dense_W": f("loc_dense_W"),
        "ta_W": f("ta_W"),
        "ta_b": f("ta_b").reshape(1, 1),
        "self_memory_W": (f("self_memory_W") * WS).astype(bf),
        "self_query_W": f("self_query_W").astype(bf),
        "self_v_W": f("self_v_W"),
    }


def kernel(**inputs) -> np.ndarray:
    f = lambda k: np.ascontiguousarray(np.asarray(inputs[k], dtype=np.float32))
    bf = ml_dtypes.bfloat16
    f8 = ml_dtypes.float8_e4m3
    rep = {
        "memory_W": (f("memory_W") * WS).astype(bf),
        "query_W": f("query_W").astype(bf),
        "v_W": f("v_W"),
        "loc_conv_W": f("loc_conv_W"),
        "loc_dense_W": f("loc_dense_W"),
        "ta_W": f("ta_W"),
        "ta_b": f("ta_b").reshape(1, 1),
        "self_memory_W": (f("self_memory_W") * WS).astype(bf),
        "self_query_W": f("self_query_W").astype(bf),
        "self_v_W": f("self_v_W"),
    }
    mem_t = np.ascontiguousarray(f("memory").transpose(0, 2, 1).astype(f8))
    smem_t = np.ascontiguousarray(f("self_memory").transpose(0, 2, 1).astype(f8))
    aw_bf = f("attention_weights").astype(bf)
    awc = f("attention_weights_cum")
    awc_bf = awc.astype(bf)
    q = f("query")
    alpha = f("alpha")
    u = f("u")
    in_maps = []
    for i in range(NCORES):
        sl = slice(i * BL, (i + 1) * BL)
        m = dict(rep)
        m["query"] = q[sl]
        m["memory"] = mem_t[sl]
        m["self_memory"] = smem_t[sl]
        m["attention_weights"] = aw_bf[sl]
        m["attention_weights_cum"] = awc[sl]
        m["awc_bf"] = awc_bf[sl]
        m["alpha"] = alpha[sl]
        m["u"] = u[sl]
        in_maps.append(m)
    global LAST_RESULT
    res = run_bass_kernel_spmd(
        _get_nc(), in_maps, core_ids=list(range(NCORES)), **RUN_KWARGS
    )
    LAST_RESULT = res
    return np.concatenate([res.results[i]["out"] for i in range(NCORES)], axis=0)
